# revision 14
# baseline (speedup 1.0000x reference)
"""ClinicalGAT Trainium2 kernel: 3 GAT layers + mean-pool + slot-head MLPs.

Strategy (8-core SPMD, graph-parallel over destination nodes):
  - Nodes are partitioned contiguously across the 8 cores (1250 each).
  - Per layer: each core computes h_aug = h_prev @ [W | v_src | v_dst] for its
    node shard (PE, node-major via PE-transposed lhsT tiles), writes a bf16
    feature table shard + f32 attention-logit ("al") shard to HBM, and the
    shards are AllGather'd (chunked, overlapping with the matmuls).
  - Attention: edges are pre-sorted by destination host-side and padded so each
    128-edge tile maps to exactly one 128-destination tile.  Per edge chunk:
    dma_gather pulls source-node feature rows (bf16) and src/dst al rows (f32);
    softmax weights w = exp(leaky_relu(al_src+al_dst)) (no max-subtraction --
    logits are bounded); a selection matrix A[e, j] = (dstloc[e] == j) is built
    on DVE, scaled per-head by w, and PE matmuls A_w.T @ gathered_h accumulate
    the weighted message sums per destination tile in PSUM.  The softmax
    denominator comes from an extra matmul w.T @ A into the same PSUM group.
  - Epilogue divides by the denominator, adds bias, applies ELU.
  - Mean-pool is a matmul against a host-built (1/count-scaled) selection
    matrix, AllReduce across cores, and the slot-head MLPs are three small
    block-diagonal matmuls.
"""

import math
import numpy as np

P = 128
CH = 8  # edge tiles per gather chunk


# ---------------------------------------------------------------- host side --

def _superchunks(ndt, npc):
    """Groups of up to 4 node tiles; returns [(d0, d1, n_rows)]."""
    out = []
    d = 0
    while d < ndt:
        d1 = min(d + 4, ndt)
        rows = min(d1 * P, npc) - d * P
        out.append((d, d1, rows))
        d = d1
    return out


def _remap(node, npc, cores, scs):
    """Map global node id -> row in the chunk-allgathered table layout."""
    c = node // npc
    l = node % npc
    base = 0
    for (d0, d1, rows) in scs:
        lo, hi = d0 * P, d0 * P + rows
        if lo <= l < hi:
            return base + c * rows + (l - lo)
        base += cores * rows
    raise AssertionError


def _preprocess(edge_index, n_nodes, cores):
    npc = n_nodes // cores
    ndt = math.ceil(npc / P)
    scs = _superchunks(ndt, npc)

    loop = np.arange(n_nodes, dtype=np.int64)
    src = np.concatenate([edge_index[0].astype(np.int64), loop])
    dst = np.concatenate([edge_index[1].astype(np.int64), loop])
    order = np.argsort(dst, kind="stable")
    src, dst = src[order], dst[order]

    remap_tab = np.array([_remap(n, npc, cores, scs) for n in range(n_nodes)],
                         dtype=np.int64)

    # split per (core, dst-tile)
    per = [[None] * ndt for _ in range(cores)]
    for c in range(cores):
        lo, hi = c * npc, (c + 1) * npc
        m = (dst >= lo) & (dst < hi)
        s_c, d_c = src[m], dst[m] - lo
        for t in range(ndt):
            mt = (d_c >= t * P) & (d_c < min((t + 1) * P, npc))
            per[c][t] = (s_c[mt], d_c[mt] - t * P)

    slot_tiles = [max(math.ceil(max(len(per[c][t][0]), 1) / P) for c in range(cores))
                  for t in range(ndt)]
    nt = sum(slot_tiles)
    nt_pad = math.ceil(nt / CH) * CH
    slot_tiles[-1] += nt_pad - nt  # trailing pad tiles extend the last dst tile
    nt = nt_pad
    ne = nt * P

    tile2dst, first, last = [], [], []
    for t in range(ndt):
        for k in range(slot_tiles[t]):
            tile2dst.append(t)
            first.append(k == 0)
            last.append(k == slot_tiles[t] - 1)

    srcw = np.zeros((cores, ne), dtype=np.int64)
    dstw = np.zeros((cores, ne), dtype=np.int64)
    dstloc = np.full((cores, ne), -1.0, dtype=np.float64)
    for c in range(cores):
        pos = 0
        for t in range(ndt):
            s_t, dl_t = per[c][t]
            k = len(s_t)
            srcw[c, pos:pos + k] = remap_tab[s_t]
            dstw[c, pos:pos + k] = remap_tab[c * npc + t * P + dl_t]
            dstloc[c, pos:pos + k] = dl_t
            pos += slot_tiles[t] * P
    return dict(npc=npc, ndt=ndt, scs=scs, nt=nt, ne=ne,
                tile2dst=tile2dst, first=first, last=last,
                srcw=srcw, dstw=dstw, dstloc=dstloc)


def _wrap_idx(idx):
    """[NE] -> wrapped int16 [128, NE/16] (16-partition wrap, replicated x8)."""
    ne = idx.shape[0]
    assert ne % 16 == 0
    a = idx.reshape(ne // 16, 16).T.astype(np.int16)
    return np.ascontiguousarray(np.tile(a, (8, 1)))


def _aug_weights(g, heads, din, dout):
    W = np.asarray(g["W"], np.float32)
    a_src = np.asarray(g["a_src"], np.float32)
    a_dst = np.asarray(g["a_dst"], np.float32)
    Wr = W.reshape(din, heads, dout)
    vsrc = np.einsum("fhd,hd->fh", Wr, a_src).astype(np.float32)
    vdst = np.einsum("fhd,hd->fh", Wr, a_dst).astype(np.float32)
    Waug = np.concatenate([W, vsrc, vdst], axis=1)
    alpad = np.zeros((din, 64 - 2 * heads), np.float32)
    Waug = np.concatenate([Waug, alpad], axis=1)
    kt = math.ceil(din / P)
    pad = np.zeros((kt * P - din, Waug.shape[1]), np.float32)
    Waug = np.concatenate([Waug, pad], 0)
    return np.ascontiguousarray(Waug.reshape(kt, P, Waug.shape[1]))


def _mlp_weights(heads_params):
    """Three combined matrices for the 8 slot-head MLPs.

    m1 = relu(g @ W1) [64, s1tot]; m2 = g1 @ W2 (relu on ternary slice);
    out = m2' @ W3.  W2/W3 are block-diagonal (identity passthrough for
    2-layer heads in W3).
    """
    sizes1 = [np.asarray(l[0][0]).shape[1] for l in heads_params]  # first-layer widths
    s1tot = sum(sizes1)
    W1 = np.zeros((P, s1tot), np.float32)
    col = 0
    for l, s in zip(heads_params, sizes1):
        W1[:np.asarray(l[0][0]).shape[0], col:col + s] = np.asarray(l[0][0], np.float32)
        col += s

    # second stage: per head either final (2-layer head) or middle (3-layer).
    # 3-layer heads are placed FIRST along the m2 feature axis so the relu
    # slice starts at partition 0 (partition offsets must be multiples of 32).
    sizes2 = [np.asarray(l[1][0]).shape[1] for l in heads_params]
    s2tot = sum(sizes2)
    order = [i for i, l in enumerate(heads_params) if len(l) == 3] + \
            [i for i, l in enumerate(heads_params) if len(l) != 3]
    row_off = np.cumsum([0] + sizes1)  # into m1 (head order)
    c2_off = {}
    c = 0
    for i in order:
        c2_off[i] = c
        c += sizes2[i]
    W2 = np.zeros((s1tot, s2tot), np.float32)
    relu_cols = []
    for i, l in enumerate(heads_params):
        W2[row_off[i]:row_off[i] + sizes1[i],
           c2_off[i]:c2_off[i] + sizes2[i]] = np.asarray(l[1][0], np.float32)
        if len(l) == 3:
            relu_cols.append((c2_off[i], c2_off[i] + sizes2[i]))

    sizes3 = [(np.asarray(l[2][0]).shape[1] if len(l) == 3 else s2)
              for l, s2 in zip(heads_params, sizes2)]
    s3tot = sum(sizes3)
    c3_off = np.cumsum([0] + sizes3)  # output stays in head order
    W3 = np.zeros((s2tot, s3tot), np.float32)
    for i, l in enumerate(heads_params):
        r, c = c2_off[i], c3_off[i]
        if len(l) == 3:
            W3[r:r + sizes2[i], c:c + sizes3[i]] = np.asarray(l[2][0], np.float32)
        else:
            W3[r:r + sizes2[i], c:c + sizes3[i]] = np.eye(sizes2[i], dtype=np.float32)

    kt2 = math.ceil(s1tot / P)
    W2p = np.zeros((kt2, P, s2tot), np.float32)
    for k in range(kt2):
        W2p[k, :min(P, s1tot - k * P), :] = W2[k * P:(k + 1) * P, :]
    W3p = np.zeros((P, s3tot), np.float32)
    W3p[:s2tot] = W3
    # relu rows of the m2 intermediate (feature-major partitions)
    assert relu_cols, "expected at least one 3-layer head"
    rlo = min(a for a, _ in relu_cols)
    rhi = max(b for _, b in relu_cols)
    for l in heads_params:
        for _, b, _ in l:
            assert not np.any(np.asarray(b)), "nonzero MLP bias unsupported"
    return W1, W2p, W3p, s1tot, s2tot, s3tot, (rlo, rhi)


# ------------------------------------------------------------- program side --

def _build_program(meta, cores, cfgs, n_graphs, mlp_shapes):
    """Emit the full Tile program; returns (nc, input tensor names)."""
    from concourse import bass, mybir, tile
    from concourse import bacc
    from concourse.masks import make_identity
    from contextlib import ExitStack

    dt = mybir.dt
    f32, bf16, i16 = dt.float32, dt.bfloat16, dt.int16
    OP = mybir.AluOpType
    ACT = mybir.ActivationFunctionType

    npc, ndt, nt, ne = meta["npc"], meta["ndt"], meta["nt"], meta["ne"]
    scs = meta["scs"]
    tile2dst, tfirst, tlast = meta["tile2dst"], meta["first"], meta["last"]
    nchunks = nt // CH
    s1tot, s2tot, s3tot, (rlo, rhi) = mlp_shapes

    nc = bacc.Bacc("TRN2", target_bir_lowering=False, debug=False,
                   num_devices=cores)
    groups = [list(range(cores))]

    # ---- external inputs
    kt1 = cfgs[0]["kt"]
    xT_d = nc.dram_tensor("xT", [kt1, ndt, P, P], f32, kind="ExternalInput")
    w_d = [nc.dram_tensor(f"w{i+1}", [c["kt"], P, c["fout"] + 64], f32,
                          kind="ExternalInput") for i, c in enumerate(cfgs)]
    srcw_d = nc.dram_tensor("srcw", [P, ne // 16], i16, kind="ExternalInput")
    dstw_d = nc.dram_tensor("dstw", [P, ne // 16], i16, kind="ExternalInput")
    dstloc_d = nc.dram_tensor("dstloc", [P, nt], bf16, kind="ExternalInput")
    iota_d = nc.dram_tensor("iota", [P, P], bf16, kind="ExternalInput")
    spool_d = nc.dram_tensor("spool", [ndt, P, n_graphs], f32, kind="ExternalInput")
    wm1_d = nc.dram_tensor("wm1", [P, s1tot], f32, kind="ExternalInput")
    kt2 = math.ceil(s1tot / P)
    wm2_d = nc.dram_tensor("wm2", [kt2, P, s2tot], f32, kind="ExternalInput")
    wm3_d = nc.dram_tensor("wm3", [P, s3tot], f32, kind="ExternalInput")
    out_d = nc.dram_tensor("out_T", [s3tot, n_graphs], f32, kind="ExternalOutput")

    in_names = ["xT", "w1", "w2", "w3", "srcw", "dstw", "dstloc", "iota",
                "spool", "wm1", "wm2", "wm3"]

    with tile.TileContext(nc) as tc, ExitStack() as ctx:
        dram = ctx.enter_context(tc.tile_pool(name="dram", bufs=1, space="DRAM"))
        n_total = npc * cores
        tables = [dram.tile([n_total, c["fout"]], bf16, tag=f"tab{i}", name=f"tab{i}")
                  for i, c in enumerate(cfgs)]
        altabs = [dram.tile([n_total, 64], f32, tag=f"al{i}", name=f"al{i}")
                  for i, c in enumerate(cfgs)]
        shards = [dram.tile([npc, c["fout"]], bf16, tag=f"shard{i}", name=f"shard{i}")
                  for i, c in enumerate(cfgs)]
        alshards = [dram.tile([npc, 64], f32, tag=f"alsh{i}", name=f"alsh{i}")
                    for i, c in enumerate(cfgs)]
        g_in = dram.tile([n_graphs, P], f32, tag="g_in")
        g_out = dram.tile([n_graphs, P], f32, tag="g_out")

        cpool = ctx.enter_context(tc.tile_pool(name="const", bufs=1))
        ident = cpool.tile([P, P], f32, tag="ident")
        make_identity(nc, ident[:])
        iota_sb = cpool.tile([P, P], bf16, tag="iota")
        nc.sync.dma_start(iota_sb[:], iota_d.ap())
        srcw = cpool.tile([P, ne // 16], i16, tag="srcw")
        nc.sync.dma_start(srcw[:], srcw_d.ap())
        dstw = cpool.tile([P, ne // 16], i16, tag="dstw")
        nc.sync.dma_start(dstw[:], dstw_d.ap())
        dstloc = cpool.tile([P, nt], bf16, tag="dstloc")
        nc.sync.dma_start(dstloc[:], dstloc_d.ap())
        spool_sb = cpool.tile([P, ndt, n_graphs], f32, tag="spool")
        nc.sync.dma_start(spool_sb[:], spool_d.ap().rearrange("d p g -> p d g"))
        wm1_sb = cpool.tile([P, s1tot], f32, tag="wm1")
        nc.sync.dma_start(wm1_sb[:], wm1_d.ap())
        wm2_sb = cpool.tile([P, kt2, s2tot], f32, tag="wm2")
        nc.sync.dma_start(wm2_sb[:], wm2_d.ap().rearrange("k p n -> p k n"))
        wm3_sb = cpool.tile([P, s3tot], f32, tag="wm3")
        nc.sync.dma_start(wm3_sb[:], wm3_d.ap())

        hpool = ctx.enter_context(tc.tile_pool(name="hsb", bufs=1))
        hsb = [hpool.tile([P, ndt, c["fout"]], f32, tag=f"hsb{i}", name=f"hsb{i}")
               for i, c in enumerate(cfgs)]

        for L, c in enumerate(cfgs):
            h, fout, ktl = c["h"], c["fout"], c["kt"]
            naug = fout + 64
            row = fout  # bf16 table row
            with tc.tile_pool(name=f"stg{L}", bufs=2) as sp, \
                 tc.tile_pool(name=f"stgw{L}", bufs=1) as swp, \
                 tc.tile_pool(name=f"stgp{L}", bufs=2, space="PSUM") as spp:
                # ---------- stage: h_aug matmuls + shard writes + allgather
                wsb_l = swp.tile([P, ktl, fout + 64], f32, tag="wsb_l",
                                 name=f"wsb_l{L}")
                nc.sync.dma_start(wsb_l[:], w_d[L].ap().rearrange("k p n -> p k n"))
                if L == 0:
                    xsb = swp.tile([P, ktl, ndt, P], f32, tag="xsb")
                    nc.sync.dma_start(
                        xsb[:], xT_d.ap().rearrange("k d p q -> p k d q"))
                rowbase = 0
                for (d0, d1, rows) in scs:
                    for d in range(d0, d1):
                        dn = min(npc - d * P, P)
                        if L == 0:
                            lhs = [xsb[:, k, d, :] for k in range(ktl)]
                        else:
                            lt = sp.tile([P, ktl, P], f32, tag="lhs")
                            for k in range(ktl):
                                pt = spp.tile([P, P], f32, tag="ptr")
                                nc.tensor.transpose(
                                    out=pt[:], identity=ident[:],
                                    in_=hsb[L - 1][:, d, k * P:(k + 1) * P])
                                nc.vector.tensor_copy(lt[:, k, :], pt[:])
                            lhs = [lt[:, k, :] for k in range(ktl)]
                        nsplits = []
                        c0 = 0
                        while c0 < naug:
                            c1 = min(c0 + 512, naug)
                            if c0 < fout < c1:
                                c1 = fout
                            nsplits.append((c0, c1))
                            c0 = c1
                        for (c0, c1) in nsplits:
                            ps = spp.tile([P, 512], f32, tag="pstage")
                            for k in range(ktl):
                                nc.tensor.matmul(
                                    ps[:, :c1 - c0], lhsT=lhs[k],
                                    rhs=wsb_l[:, k, c0:c1],
                                    start=(k == 0), stop=(k == ktl - 1))
                            if c1 <= fout:
                                st = sp.tile([P, 512], bf16, tag="stb")
                                nc.vector.tensor_copy(st[:, :c1 - c0], ps[:, :c1 - c0])
                                nc.sync.dma_start(
                                    shards[L][d * P:d * P + dn, c0:c1],
                                    st[:dn, :c1 - c0])
                            else:
                                sa = sp.tile([P, 64], f32, tag="sta")
                                nc.vector.tensor_copy(sa[:, :], ps[:, :64])
                                nc.sync.dma_start(
                                    alshards[L][d * P:d * P + dn, :],
                                    sa[:dn, :])
                    # chunked allgather of this superchunk
                    r0, r1 = d0 * P, d0 * P + rows
                    nc.gpsimd.collective_compute(
                        "AllGather", mybir.AluOpType.bypass, groups,
                        ins=[shards[L][r0:r1, :].opt()],
                        outs=[tables[L][rowbase:rowbase + cores * rows, :].opt()])
                    nc.gpsimd.collective_compute(
                        "AllGather", mybir.AluOpType.bypass, groups,
                        ins=[alshards[L][r0:r1, :].opt()],
                        outs=[altabs[L][rowbase:rowbase + cores * rows, :].opt()])
                    rowbase += cores * rows

            # ---------- attention
            with tc.tile_pool(name=f"att{L}", bufs=2) as ap, \
                 tc.tile_pool(name=f"attw{L}", bufs=2) as wp, \
                 tc.tile_pool(name=f"aw{L}", bufs=3) as awp, \
                 tc.tile_pool(name=f"attp{L}", bufs=2, space="PSUM") as pp, \
                 tc.tile_pool(name=f"attps{L}", bufs=2, space="PSUM") as pps, \
                 tc.tile_pool(name=f"attps2{L}", bufs=1, space="PSUM") as pps1:
                npacks = math.ceil(h * P / 512)
                packs = None
                den = None
                for ch in range(nchunks):
                    i0 = ch * CH * 8  # wrapped idx col offset
                    g = ap.tile([P, CH, row], bf16, tag="g")
                    nc.gpsimd.dma_gather(
                        g[:], tables[L][:, :], srcw[:, i0:i0 + CH * 8],
                        num_idxs=CH * P, num_idxs_reg=CH * P, elem_size=row)
                    asg = ap.tile([P, CH, 64], f32, tag="asg")
                    nc.gpsimd.dma_gather(
                        asg[:], altabs[L][:, :], srcw[:, i0:i0 + CH * 8],
                        num_idxs=CH * P, num_idxs_reg=CH * P, elem_size=64)
                    adg = ap.tile([P, CH, 64], f32, tag="adg")
                    nc.gpsimd.dma_gather(
                        adg[:], altabs[L][:, :], dstw[:, i0:i0 + CH * 8],
                        num_idxs=CH * P, num_idxs_reg=CH * P, elem_size=64)
                    wf = wp.tile([P, CH, h], f32, tag="wf")
                    nc.vector.tensor_tensor(
                        out=wf[:], in0=asg[:, :, 0:h], in1=adg[:, :, h:2 * h],
                        op=OP.add)
                    lrt = wp.tile([P, CH, h], f32, tag="lrt")
                    nc.vector.tensor_scalar(
                        out=lrt[:], in0=wf[:], scalar1=0.2, scalar2=None,
                        op0=OP.mult)
                    nc.vector.tensor_tensor(
                        out=wf[:], in0=wf[:], in1=lrt[:], op=OP.max)
                    nc.scalar.activation(wf[:], wf[:], ACT.Exp)
                    wb = wp.tile([P, CH, h], bf16, tag="wb")
                    nc.vector.tensor_copy(wb[:], wf[:])
                    Ab = wp.tile([P, CH, P], bf16, tag="Ab")
                    nc.vector.tensor_tensor(
                        out=Ab[:],
                        in0=dstloc[:, ch * CH:(ch + 1) * CH].unsqueeze(2)
                            .to_broadcast([P, CH, P]),
                        in1=iota_sb[:].unsqueeze(1).to_broadcast([P, CH, P]),
                        op=OP.is_equal)
                    for t in range(CH):
                        Aw = awp.tile([P, h, P], bf16, tag="Aw")
                        nc.vector.tensor_tensor(
                            out=Aw[:],
                            in0=Ab[:, t, :].unsqueeze(1).to_broadcast([P, h, P]),
                            in1=wb[:, t, :].unsqueeze(2).to_broadcast([P, h, P]),
                            op=OP.mult)
                        gt = ch * CH + t
                        dtile = tile2dst[gt]
                        if tfirst[gt]:
                            packs = [pp.tile([P, 512], f32, tag=f"pk{i}", name=f"pk{i}")
                                     for i in range(npacks)]
                            den = pps.tile([h, P], f32, tag="den")
                        for hh in range(h):
                            pk = packs[(hh * P) // 512]
                            off = (hh * P) % 512
                            # start=True resets has_written for the whole
                            # PSUM tile -- only the first write to each pack
                            # may set it.
                            nc.tensor.matmul(
                                pk[:, off:off + P], lhsT=Aw[:, hh, :],
                                rhs=g[:, t, hh * P:(hh + 1) * P],
                                start=(tfirst[gt] and off == 0),
                                stop=tlast[gt],
                                skip_group_check=True)
                        nc.tensor.matmul(
                            den[:, :], lhsT=wb[:, t, :], rhs=Ab[:, t, :],
                            start=tfirst[gt], stop=tlast[gt],
                            skip_group_check=True)
                        if tlast[gt]:
                            # epilogue for dst tile `dtile`
                            dsb = wp.tile([h, P], f32, tag="dsb")
                            nc.vector.tensor_copy(dsb[:], den[:, :])
                            dtp = pps1.tile([P, h], f32, tag="dtp")
                            nc.tensor.transpose(out=dtp[:], in_=dsb[:],
                                                identity=ident[:h, :h])
                            r = wp.tile([P, h], f32, tag="rcp")
                            nc.vector.tensor_scalar(
                                out=r[:], in0=dtp[:], scalar1=1e-16,
                                scalar2=None, op0=OP.add)
                            nc.vector.reciprocal(r[:], r[:])
                            hv = hsb[L][:, dtile, :]
                            for hh in range(h):
                                pk = packs[(hh * P) // 512]
                                off = (hh * P) % 512
                                nc.vector.tensor_scalar(
                                    out=hv[:, hh * P:(hh + 1) * P],
                                    in0=pk[:, off:off + P],
                                    scalar1=r[:, hh:hh + 1], scalar2=None,
                                    op0=OP.mult)
                            tmp = wp.tile([P, fout], f32, tag="elu")
                            nc.vector.tensor_scalar(
                                out=tmp[:], in0=hv, scalar1=0.0, scalar2=None,
                                op0=OP.min)
                            nc.scalar.activation(tmp[:], tmp[:], ACT.Exp)
                            nc.vector.tensor_scalar(
                                out=tmp[:], in0=tmp[:], scalar1=-1.0,
                                scalar2=None, op0=OP.add)
                            nc.vector.tensor_tensor(
                                out=hv, in0=hv, in1=tmp[:], op=OP.max)

        # ---------- mean pool + heads
        with tc.tile_pool(name="head", bufs=1) as hp, \
             tc.tile_pool(name="headp", bufs=1, space="PSUM") as hpp:
            psg = hpp.tile([n_graphs, P], f32, tag="psg")
            for d in range(ndt):
                nc.tensor.matmul(psg[:], lhsT=spool_sb[:, d, :],
                                 rhs=hsb[2][:, d, :],
                                 start=(d == 0), stop=(d == ndt - 1))
            gsb = hp.tile([n_graphs, P], f32, tag="gsb")
            nc.vector.tensor_copy(gsb[:], psg[:])
            nc.sync.dma_start(g_in[:, :], gsb[:])
            nc.gpsimd.collective_compute(
                "AllReduce", mybir.AluOpType.add, groups,
                ins=[g_in[:, :].opt()], outs=[g_out[:, :].opt()])
            gfull = hp.tile([n_graphs, P], f32, tag="gfull")
            nc.sync.dma_start(gfull[:], g_out[:, :])
            pgt = hpp.tile([P, n_graphs], f32, tag="pgt")
            nc.tensor.transpose(out=pgt[:], in_=gfull[:],
                                identity=ident[:n_graphs, :n_graphs])
            gT = hp.tile([P, n_graphs], f32, tag="gT")
            nc.vector.tensor_copy(gT[:], pgt[:])

            m1 = hp.tile([P, kt2, n_graphs], f32, tag="m1")
            nc.vector.memset(m1[:], 0.0)
            for k in range(kt2):
                mwid = min(P, s1tot - k * P)
                pm = hpp.tile([P, n_graphs], f32, tag="pm1")
                nc.tensor.matmul(pm[:mwid, :], lhsT=wm1_sb[:, k * P:k * P + mwid],
                                 rhs=gT[:], start=True, stop=True)
                nc.scalar.activation(m1[:mwid, k, :], pm[:mwid, :], ACT.Relu)
            pm2 = hpp.tile([s2tot, n_graphs], f32, tag="pm2")
            for k in range(kt2):
                nc.tensor.matmul(pm2[:], lhsT=wm2_sb[:, k, :], rhs=m1[:, k, :],
                                 start=(k == 0), stop=(k == kt2 - 1))
            m2 = hp.tile([P, n_graphs], f32, tag="m2")
            nc.vector.memset(m2[:], 0.0)
            if rlo > 0:
                nc.vector.tensor_copy(m2[0:rlo, :], pm2[0:rlo, :])
            nc.scalar.activation(m2[rlo:rhi, :], pm2[rlo:rhi, :], ACT.Relu)
            if rhi < s2tot:
                nc.vector.tensor_copy(m2[rhi:s2tot, :], pm2[rhi:s2tot, :])
            pm3 = hpp.tile([s3tot, n_graphs], f32, tag="pm3")
            nc.tensor.matmul(pm3[:], lhsT=wm3_sb[:], rhs=m2[:], start=True,
                             stop=True)
            osb = hp.tile([s3tot, n_graphs], f32, tag="osb")
            nc.vector.tensor_copy(osb[:], pm3[:])
            nc.sync.dma_start(out_d.ap(), osb[:])

    nc.compile()
    return nc, in_names


# ------------------------------------------------------------------ driver --

def _host_inputs(x, edge_index, batch, params, meta, cfgs, cores):
    """Per-core input tensors for the program."""
    import ml_dtypes

    x = np.asarray(x, np.float32)
    batch = np.asarray(batch)
    n_nodes = x.shape[0]
    n_graphs = 64
    npc = n_nodes // cores
    ndt = meta["ndt"]

    g1, g2, g3 = params["gat1"], params["gat2"], params["gat3"]
    fin1 = x.shape[1]
    w1 = _aug_weights(g1, 8, fin1, 128)
    w2 = _aug_weights(g2, 4, 1024, 128)
    w3 = _aug_weights(g3, 1, 512, 128)
    for g in (g1, g2, g3):
        assert not np.any(np.asarray(g["b"])), "nonzero GAT bias unsupported"
    W1m, W2m, W3m, *_ = _mlp_weights(params["heads"])

    cnts = np.bincount(batch, minlength=n_graphs).astype(np.float64)
    cnts = np.maximum(cnts, 1.0)
    spool = np.zeros((cores, ndt, P, n_graphs), np.float32)
    inv = 1.0 / cnts[batch]
    for c in range(cores):
        for d in range(ndt):
            pn = min(P, npc - d * P)
            n0 = c * npc + d * P
            spool[c, d, np.arange(pn), batch[n0:n0 + pn]] = inv[n0:n0 + pn]

    kt1 = cfgs[0]["kt"]
    xT = np.zeros((cores, kt1, ndt, P, P), np.float32)
    for c in range(cores):
        xc = x[c * npc:(c + 1) * npc]
        for k in range(kt1):
            for d in range(ndt):
                blk = xc[d * P:(d + 1) * P, k * P:(k + 1) * P]
                xT[c, k, d, :blk.shape[1], :blk.shape[0]] = blk.T

    iota = np.tile(np.arange(P, dtype=np.float32), (P, 1))

    in_maps = []
    for c in range(cores):
        in_maps.append({
            "xT": np.ascontiguousarray(xT[c]),
            "w1": w1, "w2": w2, "w3": w3,
            "srcw": _wrap_idx(meta["srcw"][c]),
            "dstw": _wrap_idx(meta["dstw"][c]),
            "dstloc": np.ascontiguousarray(
                meta["dstloc"][c].reshape(meta["nt"], P).T.astype(ml_dtypes.bfloat16)),
            "iota": iota.astype(ml_dtypes.bfloat16),
            "spool": np.ascontiguousarray(spool[c]),
            "wm1": W1m, "wm2": W2m, "wm3": W3m,
        })
    return in_maps


def _run(x, edge_index, batch, params, cores=8, trace=False):
    from concourse.bass_utils import run_bass_kernel_spmd

    x = np.asarray(x, np.float32)
    edge_index = np.asarray(edge_index)
    batch = np.asarray(batch)
    n_nodes = x.shape[0]
    n_graphs = 64

    meta = _preprocess(edge_index, n_nodes, cores)
    fin1 = x.shape[1]
    cfgs = [
        dict(h=8, fout=1024, kt=math.ceil(fin1 / P)),
        dict(h=4, fout=512, kt=8),
        dict(h=1, fout=128, kt=4),
    ]
    _, _, _, s1tot, s2tot, s3tot, relu_rows = _mlp_weights(params["heads"])
    nc, _ = _build_program(meta, cores, cfgs, n_graphs,
                           (s1tot, s2tot, s3tot, relu_rows))
    in_maps = _host_inputs(x, edge_index, batch, params, meta, cfgs, cores)
    res = run_bass_kernel_spmd(nc, in_maps, core_ids=list(range(cores)),
                               trace=trace)
    out = res.results[0]["out_T"]  # [18, 64]
    return np.ascontiguousarray(out.T.astype(np.float32)), res


def kernel(x, edge_index, batch, params):
    out, _ = _run(x, edge_index, batch, params)
    return out


# revision 24
# speedup vs baseline: 61.0291x; 61.0291x over previous
"""ClinicalGAT Trainium2 kernel: 3 GAT layers + mean-pool + slot-head MLPs.

Strategy (8-core SPMD, graph-parallel over destination nodes):
  - Nodes are partitioned contiguously across the 8 cores (1250 each).
  - Per layer: each core computes h_aug = h_prev @ [W | v_src | v_dst] for its
    node shard (PE, node-major via PE-transposed lhsT tiles), writes a bf16
    feature table shard + f32 attention-logit ("al") shard to HBM, and the
    shards are AllGather'd (chunked, overlapping with the matmuls).
  - Attention: edges are pre-sorted by destination host-side and padded so each
    128-edge tile maps to exactly one 128-destination tile.  Per edge chunk:
    dma_gather pulls source-node feature rows (bf16) and src/dst al rows (f32);
    softmax weights w = exp(leaky_relu(al_src+al_dst)) (no max-subtraction --
    logits are bounded); a selection matrix A[e, j] = (dstloc[e] == j) is built
    on DVE, scaled per-head by w, and PE matmuls A_w.T @ gathered_h accumulate
    the weighted message sums per destination tile in PSUM.  The softmax
    denominator comes from an extra matmul w.T @ A into the same PSUM group.
  - Epilogue divides by the denominator, adds bias, applies ELU.
  - Mean-pool is a matmul against a host-built (1/count-scaled) selection
    matrix, AllReduce across cores, and the slot-head MLPs are three small
    block-diagonal matmuls.
"""

import math
import numpy as np

P = 128
CH = 8  # edge tiles per gather chunk


# ---------------------------------------------------------------- host side --

def _superchunks(ndt, npc):
    """Groups of up to 4 node tiles; returns [(d0, d1, n_rows)]."""
    out = []
    d = 0
    while d < ndt:
        d1 = min(d + 4, ndt)
        rows = min(d1 * P, npc) - d * P
        out.append((d, d1, rows))
        d = d1
    return out


def _remap(node, npc, cores, scs):
    """Map global node id -> row in the chunk-allgathered table layout."""
    c = node // npc
    l = node % npc
    base = 0
    for (d0, d1, rows) in scs:
        lo, hi = d0 * P, d0 * P + rows
        if lo <= l < hi:
            return base + c * rows + (l - lo)
        base += cores * rows
    raise AssertionError


def _preprocess(edge_index, n_nodes, cores):
    npc = n_nodes // cores
    ndt = math.ceil(npc / P)
    scs = _superchunks(ndt, npc)

    loop = np.arange(n_nodes, dtype=np.int64)
    src = np.concatenate([edge_index[0].astype(np.int64), loop])
    dst = np.concatenate([edge_index[1].astype(np.int64), loop])
    order = np.argsort(dst, kind="stable")
    src, dst = src[order], dst[order]

    remap_tab = np.arange(n_nodes, dtype=np.int64)  # table rows = node order

    # split per (core, dst-tile)
    per = [[None] * ndt for _ in range(cores)]
    for c in range(cores):
        lo, hi = c * npc, (c + 1) * npc
        m = (dst >= lo) & (dst < hi)
        s_c, d_c = src[m], dst[m] - lo
        for t in range(ndt):
            mt = (d_c >= t * P) & (d_c < min((t + 1) * P, npc))
            per[c][t] = (s_c[mt], d_c[mt] - t * P)

    slot_tiles = [max(math.ceil(max(len(per[c][t][0]), 1) / P) for c in range(cores))
                  for t in range(ndt)]
    nt = sum(slot_tiles)
    nt_pad = math.ceil(nt / CH) * CH
    slot_tiles[-1] += nt_pad - nt  # trailing pad tiles extend the last dst tile
    nt = nt_pad
    ne = nt * P

    tile2dst, first, last = [], [], []
    for t in range(ndt):
        for k in range(slot_tiles[t]):
            tile2dst.append(t)
            first.append(k == 0)
            last.append(k == slot_tiles[t] - 1)

    srcw = np.zeros((cores, ne), dtype=np.int64)
    dstw = np.zeros((cores, ne), dtype=np.int64)
    dstloc = np.full((cores, ne), -1.0, dtype=np.float64)
    for c in range(cores):
        pos = 0
        for t in range(ndt):
            s_t, dl_t = per[c][t]
            k = len(s_t)
            srcw[c, pos:pos + k] = remap_tab[s_t]
            dstw[c, pos:pos + k] = remap_tab[c * npc + t * P + dl_t]
            dstloc[c, pos:pos + k] = dl_t
            pos += slot_tiles[t] * P
    return dict(npc=npc, ndt=ndt, scs=scs, nt=nt, ne=ne,
                tile2dst=tile2dst, first=first, last=last,
                srcw=srcw, dstw=dstw, dstloc=dstloc)


def _wrap_idx(idx):
    """[NE] -> wrapped int16 [128, NE/16] (16-partition wrap, replicated x8)."""
    ne = idx.shape[0]
    assert ne % 16 == 0
    a = idx.reshape(ne // 16, 16).T.astype(np.int16)
    return np.ascontiguousarray(np.tile(a, (8, 1)))


def _aug_weights(g, heads, din, dout):
    W = np.asarray(g["W"], np.float32)
    a_src = np.asarray(g["a_src"], np.float32)
    a_dst = np.asarray(g["a_dst"], np.float32)
    Wr = W.reshape(din, heads, dout)
    vsrc = np.einsum("fhd,hd->fh", Wr, a_src).astype(np.float32)
    vdst = np.einsum("fhd,hd->fh", Wr, a_dst).astype(np.float32)
    Waug = np.concatenate([W, vsrc, vdst], axis=1)
    kt = math.ceil(din / P)
    pad = np.zeros((kt * P - din, Waug.shape[1]), np.float32)
    Waug = np.concatenate([Waug, pad], 0)
    return np.ascontiguousarray(Waug.reshape(kt, P, Waug.shape[1]))


def _mlp_weights(heads_params):
    """Three combined matrices for the 8 slot-head MLPs.

    m1 = relu(g @ W1) [64, s1tot]; m2 = g1 @ W2 (relu on ternary slice);
    out = m2' @ W3.  W2/W3 are block-diagonal (identity passthrough for
    2-layer heads in W3).
    """
    sizes1 = [np.asarray(l[0][0]).shape[1] for l in heads_params]  # first-layer widths
    s1tot = sum(sizes1)
    W1 = np.zeros((P, s1tot), np.float32)
    col = 0
    for l, s in zip(heads_params, sizes1):
        W1[:np.asarray(l[0][0]).shape[0], col:col + s] = np.asarray(l[0][0], np.float32)
        col += s

    # second stage: per head either final (2-layer head) or middle (3-layer).
    # 3-layer heads are placed FIRST along the m2 feature axis so the relu
    # slice starts at partition 0 (partition offsets must be multiples of 32).
    sizes2 = [np.asarray(l[1][0]).shape[1] for l in heads_params]
    s2tot = sum(sizes2)
    order = [i for i, l in enumerate(heads_params) if len(l) == 3] + \
            [i for i, l in enumerate(heads_params) if len(l) != 3]
    row_off = np.cumsum([0] + sizes1)  # into m1 (head order)
    c2_off = {}
    c = 0
    for i in order:
        c2_off[i] = c
        c += sizes2[i]
    W2 = np.zeros((s1tot, s2tot), np.float32)
    relu_cols = []
    for i, l in enumerate(heads_params):
        W2[row_off[i]:row_off[i] + sizes1[i],
           c2_off[i]:c2_off[i] + sizes2[i]] = np.asarray(l[1][0], np.float32)
        if len(l) == 3:
            relu_cols.append((c2_off[i], c2_off[i] + sizes2[i]))

    sizes3 = [(np.asarray(l[2][0]).shape[1] if len(l) == 3 else s2)
              for l, s2 in zip(heads_params, sizes2)]
    s3tot = sum(sizes3)
    c3_off = np.cumsum([0] + sizes3)  # output stays in head order
    W3 = np.zeros((s2tot, s3tot), np.float32)
    for i, l in enumerate(heads_params):
        r, c = c2_off[i], c3_off[i]
        if len(l) == 3:
            W3[r:r + sizes2[i], c:c + sizes3[i]] = np.asarray(l[2][0], np.float32)
        else:
            W3[r:r + sizes2[i], c:c + sizes3[i]] = np.eye(sizes2[i], dtype=np.float32)

    kt2 = math.ceil(s1tot / P)
    W2p = np.zeros((kt2, P, s2tot), np.float32)
    for k in range(kt2):
        W2p[k, :min(P, s1tot - k * P), :] = W2[k * P:(k + 1) * P, :]
    W3p = np.zeros((P, s3tot), np.float32)
    W3p[:s2tot] = W3
    # relu rows of the m2 intermediate (feature-major partitions)
    assert relu_cols, "expected at least one 3-layer head"
    rlo = min(a for a, _ in relu_cols)
    rhi = max(b for _, b in relu_cols)
    for l in heads_params:
        for _, b, _ in l:
            assert not np.any(np.asarray(b)), "nonzero MLP bias unsupported"
    return W1, W2p, W3p, s1tot, s2tot, s3tot, (rlo, rhi)


# ------------------------------------------------------------- program side --

def _build_program(meta, cores, cfgs, n_graphs, mlp_shapes):
    """Emit the full Tile program; returns (nc, input tensor names)."""
    from concourse import bass, mybir, tile
    from concourse import bacc
    from concourse.masks import make_identity
    from contextlib import ExitStack

    dt = mybir.dt
    f32, bf16, i16 = dt.float32, dt.bfloat16, dt.int16
    OP = mybir.AluOpType
    ACT = mybir.ActivationFunctionType

    npc, ndt, nt, ne = meta["npc"], meta["ndt"], meta["nt"], meta["ne"]
    tile2dst, tfirst, tlast = meta["tile2dst"], meta["first"], meta["last"]
    s1tot, s2tot, s3tot, (rlo, rhi) = mlp_shapes
    rows_l = [1152, 640, 256]  # bf16 table row: fout | f32-bitcast al | pad

    nc = bacc.Bacc("TRN2", target_bir_lowering=False, debug=False,
                   num_devices=cores)
    groups = [list(range(cores))]

    # ---- external inputs
    kt1 = cfgs[0]["kt"]
    n_total = npc * cores
    nft = math.ceil(n_total / P)  # layer-0 stage is replicated over all nodes
    xT_d = nc.dram_tensor("xT", [kt1, nft, P, P], f32, kind="ExternalInput")
    w_d = [nc.dram_tensor(f"w{i+1}", [c["kt"], P, c["fout"] + 2 * c["h"]], f32,
                          kind="ExternalInput") for i, c in enumerate(cfgs)]
    srcw_d = nc.dram_tensor("srcw", [P, ne // 16], i16, kind="ExternalInput")
    dstw_d = nc.dram_tensor("dstw", [P, ne // 16], i16, kind="ExternalInput")
    dstloc_d = nc.dram_tensor("dstloc", [P, nt], f32, kind="ExternalInput")
    iota_d = nc.dram_tensor("iota", [P, P], bf16, kind="ExternalInput")
    spool_d = nc.dram_tensor("spool", [ndt, P, n_graphs], f32, kind="ExternalInput")
    wm1_d = nc.dram_tensor("wm1", [P, s1tot], f32, kind="ExternalInput")
    kt2 = math.ceil(s1tot / P)
    wm2_d = nc.dram_tensor("wm2", [kt2, P, s2tot], f32, kind="ExternalInput")
    wm3_d = nc.dram_tensor("wm3", [P, s3tot], f32, kind="ExternalInput")
    out_d = nc.dram_tensor("out_T", [s3tot, n_graphs], f32, kind="ExternalOutput")

    in_names = ["xT", "w1", "w2", "w3", "srcw", "dstw", "dstloc", "iota",
                "spool", "wm1", "wm2", "wm3"]

    with tile.TileContext(nc) as tc, ExitStack() as ctx:
        dram = ctx.enter_context(tc.tile_pool(name="dram", bufs=1, space="DRAM"))
        tables = [dram.tile([n_total, rows_l[i]], bf16, tag=f"tab{i}", name=f"tab{i}")
                  for i, c in enumerate(cfgs)]
        shards = [None] + [dram.tile([npc, rows_l[i]], bf16, tag=f"shard{i}",
                                     name=f"shard{i}")
                           for i, c in list(enumerate(cfgs))[1:]]
        g_in = dram.tile([n_graphs, P], f32, tag="g_in")
        g_out = dram.tile([n_graphs, P], f32, tag="g_out")

        cpool = ctx.enter_context(tc.tile_pool(name="const", bufs=1))
        ident = cpool.tile([P, P], f32, tag="ident")
        make_identity(nc, ident[:])
        iota_sb = cpool.tile([P, P], bf16, tag="iota")
        nc.sync.dma_start(iota_sb[:], iota_d.ap())
        srcw = cpool.tile([P, ne // 16], i16, tag="srcw")
        nc.sync.dma_start(srcw[:], srcw_d.ap())
        dstw = cpool.tile([P, ne // 16], i16, tag="dstw")
        nc.sync.dma_start(dstw[:], dstw_d.ap())
        dstloc = cpool.tile([P, nt], f32, tag="dstloc")
        nc.sync.dma_start(dstloc[:], dstloc_d.ap())
        spool_sb = cpool.tile([P, ndt, n_graphs], f32, tag="spool")
        nc.sync.dma_start(spool_sb[:], spool_d.ap().rearrange("d p g -> p d g"))
        wm1_sb = cpool.tile([P, s1tot], f32, tag="wm1")
        nc.sync.dma_start(wm1_sb[:], wm1_d.ap())
        wm2_sb = cpool.tile([P, kt2, s2tot], f32, tag="wm2")
        nc.sync.dma_start(wm2_sb[:], wm2_d.ap().rearrange("k p n -> p k n"))
        wm3_sb = cpool.tile([P, s3tot], f32, tag="wm3")
        nc.sync.dma_start(wm3_sb[:], wm3_d.ap())

        hpool = ctx.enter_context(tc.tile_pool(name="hsb", bufs=1))
        hsb = [hpool.tile([P, ndt, c["fout"]], f32, tag=f"hsb{i}", name=f"hsb{i}")
               for i, c in enumerate(cfgs)]

        for L, c in enumerate(cfgs):
            h, fout, ktl = c["h"], c["fout"], c["kt"]
            naug = fout + 2 * h
            with tc.tile_pool(name=f"stg{L}", bufs=2) as sp, \
                 tc.tile_pool(name=f"stgw{L}", bufs=1) as swp, \
                 tc.tile_pool(name=f"stgp{L}", bufs=2, space="PSUM") as spp:
                # ---------- stage: h_aug matmuls + shard writes + allgather
                wsb_l = swp.tile([P, ktl, naug], f32, tag="wsb_l",
                                 name=f"wsb_l{L}")
                nc.sync.dma_start(wsb_l[:], w_d[L].ap().rearrange("k p n -> p k n"))
                # walrus requires fp32r matmul operands to be explicitly
                # rounded by their producing instruction
                wsb_r = swp.tile([P, ktl, naug], dt.float32r, tag="wsb_r",
                                 name=f"wsb_r{L}")
                nc.vector.tensor_copy(wsb_r[:], wsb_l[:])
                # L0 is replicated over all node tiles (no collective); L>0
                # computes its own shard then one AllGather.
                row = rows_l[L]
                dest = tables[0] if L == 0 else shards[L]
                n_rows = n_total if L == 0 else npc
                n_dt = nft if L == 0 else ndt
                for d in range(n_dt):
                    dn = min(n_rows - d * P, P)
                    lt = sp.tile([P, ktl, P], dt.float32r, tag="lhs")
                    if L == 0:
                        lt0 = sp.tile([P, ktl, P], f32, tag="lhs0")
                        nc.sync.dma_start(
                            lt0[:], xT_d.ap()[:, d, :, :].rearrange("k p q -> p k q"))
                        nc.vector.tensor_copy(lt[:], lt0[:])
                    else:
                        for k in range(ktl):
                            pt = spp.tile([P, P], f32, tag="ptr")
                            nc.tensor.transpose(
                                out=pt[:], identity=ident[:],
                                in_=hsb[L - 1][:, d, k * P:(k + 1) * P])
                            nc.vector.tensor_copy(lt[:, k, :], pt[:])
                    lhs = [lt[:, k, :] for k in range(ktl)]
                    nsplits = []
                    c0 = 0
                    while c0 < naug:
                        c1 = min(c0 + 512, naug)
                        if c0 < fout < c1:
                            c1 = fout
                        nsplits.append((c0, c1))
                        c0 = c1
                    stf = sp.tile([P, row], bf16, tag="stb")
                    for (c0, c1) in nsplits:
                        ps = spp.tile([P, 512], f32, tag="pstage")
                        for k in range(ktl):
                            nc.tensor.matmul(
                                ps[:, :c1 - c0], lhsT=lhs[k],
                                rhs=wsb_r[:, k, c0:c1],
                                start=(k == 0), stop=(k == ktl - 1))
                        if c1 <= fout:
                            nc.scalar.activation(stf[:, c0:c1], ps[:, :c1 - c0],
                                                 ACT.Copy)
                        else:
                            # al cols: f32 bitcast into the bf16 row + zero pad
                            nc.vector.memset(stf[:, fout + 4 * h:row], 0)
                            nc.vector.tensor_copy(
                                stf[:, fout:fout + 4 * h].bitcast(f32),
                                ps[:, :2 * h])
                    nc.sync.dma_start(dest[d * P:d * P + dn, :], stf[:dn, :])
                if L > 0:
                    nc.gpsimd.collective_compute(
                        "AllGather", mybir.AluOpType.bypass, groups,
                        ins=[shards[L][:, :].opt()],
                        outs=[tables[L][:, :].opt()])

            # ---------- attention
            with tc.tile_pool(name=f"att{L}", bufs=2) as ap, \
                 tc.tile_pool(name=f"attw{L}", bufs=2) as wp, \
                 tc.tile_pool(name=f"aw{L}", bufs=3) as awp, \
                 tc.tile_pool(name=f"attp{L}", bufs=2, space="PSUM") as pp, \
                 tc.tile_pool(name=f"attps{L}", bufs=2, space="PSUM") as pps, \
                 tc.tile_pool(name=f"attps2{L}", bufs=1, space="PSUM") as pps1:
                npacks = math.ceil(h * P / 512)
                packs = None
                den = None
                row = rows_l[L]
                chl = CH  # edge tiles per gather chunk (1024 descs fits SWDGE ring)
                nchunks = nt // chl
                for ch in range(nchunks):
                    i0 = ch * chl * 8  # wrapped idx col offset
                    g = ap.tile([P, chl, row], bf16, tag="g")
                    nc.gpsimd.dma_gather(
                        g[:], tables[L][:, :], srcw[:, i0:i0 + chl * 8],
                        num_idxs=chl * P, num_idxs_reg=chl * P, elem_size=row)
                    adg = ap.tile([P, chl, P], bf16, tag="adg")
                    nc.gpsimd.dma_gather(
                        adg[:], tables[L][:, fout:fout + P],
                        dstw[:, i0:i0 + chl * 8],
                        num_idxs=chl * P, num_idxs_reg=chl * P, elem_size=P,
                        elem_step=row)
                    # al sections are f32 bitcast inside the bf16 rows
                    wf = wp.tile([P, chl, h], f32, tag="wf")
                    nc.vector.tensor_tensor(
                        out=wf[:],
                        in0=g[:, :, fout:fout + 4 * h].bitcast(f32)[:, :, 0:h],
                        in1=adg[:, :, 0:4 * h].bitcast(f32)[:, :, h:2 * h],
                        op=OP.add)
                    lrt = wp.tile([P, chl, h], f32, tag="lrt")
                    nc.vector.tensor_scalar(
                        out=lrt[:], in0=wf[:], scalar1=0.2, scalar2=None,
                        op0=OP.mult)
                    nc.vector.tensor_tensor(
                        out=wf[:], in0=wf[:], in1=lrt[:], op=OP.max)
                    nc.scalar.activation(wf[:], wf[:], ACT.Exp)
                    wb = wp.tile([P, chl, h], bf16, tag="wb")
                    nc.vector.tensor_copy(wb[:], wf[:])
                    for t in range(chl):
                        gt = ch * chl + t
                        Ab = awp.tile([P, P], bf16, tag="Ab", name="Ab")
                        nc.vector.tensor_scalar(
                            out=Ab[:], in0=iota_sb[:],
                            scalar1=dstloc[:, gt:gt + 1], scalar2=None,
                            op0=OP.is_equal)
                        Aw = awp.tile([P, h, P], bf16, tag="Aw")
                        for hh in range(h):
                            nc.vector.tensor_scalar(
                                out=Aw[:, hh, :], in0=iota_sb[:],
                                scalar1=dstloc[:, gt:gt + 1],
                                scalar2=wf[:, t, hh:hh + 1],
                                op0=OP.is_equal, op1=OP.mult)
                        dtile = tile2dst[gt]
                        if tfirst[gt]:
                            packs = [pp.tile([P, 512], f32, tag=f"pk{i}", name=f"pk{i}")
                                     for i in range(npacks)]
                            den = pps.tile([h, P], f32, tag="den")
                        for hh in range(h):
                            pk = packs[(hh * P) // 512]
                            off = (hh * P) % 512
                            # start=True resets has_written for the whole
                            # PSUM tile -- only the first write to each pack
                            # may set it.
                            nc.tensor.matmul(
                                pk[:, off:off + P], lhsT=Aw[:, hh, :],
                                rhs=g[:, t, hh * P:(hh + 1) * P],
                                start=(tfirst[gt] and off == 0),
                                stop=tlast[gt],
                                skip_group_check=True)
                        nc.tensor.matmul(
                            den[:, :], lhsT=wb[:, t, :], rhs=Ab[:],
                            start=tfirst[gt], stop=tlast[gt],
                            skip_group_check=True)
                        if tlast[gt]:
                            # epilogue for dst tile `dtile`
                            dsb = wp.tile([h, P], f32, tag="dsb")
                            nc.vector.tensor_copy(dsb[:], den[:, :])
                            dtp = pps1.tile([P, h], f32, tag="dtp")
                            nc.tensor.transpose(out=dtp[:], in_=dsb[:],
                                                identity=ident[:h, :h])
                            r = wp.tile([P, h], f32, tag="rcp")
                            nc.vector.tensor_scalar(
                                out=r[:], in0=dtp[:], scalar1=1e-16,
                                scalar2=None, op0=OP.add)
                            nc.vector.reciprocal(r[:], r[:])
                            hv = hsb[L][:, dtile, :]
                            for hh in range(h):
                                pk = packs[(hh * P) // 512]
                                off = (hh * P) % 512
                                nc.scalar.activation(
                                    hv[:, hh * P:(hh + 1) * P],
                                    pk[:, off:off + P], ACT.Copy,
                                    scale=r[:, hh:hh + 1])
                            # elu(v) = max(v, min(exp(v), 1) - 1)
                            tmp = wp.tile([P, fout], f32, tag="elu")
                            nc.scalar.activation(tmp[:], hv, ACT.Exp)
                            nc.vector.tensor_scalar(
                                out=tmp[:], in0=tmp[:], scalar1=1.0,
                                scalar2=-1.0, op0=OP.min, op1=OP.add)
                            nc.vector.tensor_tensor(
                                out=hv, in0=hv, in1=tmp[:], op=OP.max)

        # ---------- mean pool + heads
        with tc.tile_pool(name="head", bufs=1) as hp, \
             tc.tile_pool(name="headp", bufs=1, space="PSUM") as hpp:
            psg = hpp.tile([n_graphs, P], f32, tag="psg")
            for d in range(ndt):
                nc.tensor.matmul(psg[:], lhsT=spool_sb[:, d, :],
                                 rhs=hsb[2][:, d, :],
                                 start=(d == 0), stop=(d == ndt - 1))
            gsb = hp.tile([n_graphs, P], f32, tag="gsb")
            nc.vector.tensor_copy(gsb[:], psg[:])
            nc.sync.dma_start(g_in[:, :], gsb[:])
            nc.gpsimd.collective_compute(
                "AllReduce", mybir.AluOpType.add, groups,
                ins=[g_in[:, :].opt()], outs=[g_out[:, :].opt()])
            gfull = hp.tile([n_graphs, P], f32, tag="gfull")
            nc.sync.dma_start(gfull[:], g_out[:, :])
            pgt = hpp.tile([P, n_graphs], f32, tag="pgt")
            nc.tensor.transpose(out=pgt[:], in_=gfull[:],
                                identity=ident[:n_graphs, :n_graphs])
            gT = hp.tile([P, n_graphs], f32, tag="gT")
            nc.vector.tensor_copy(gT[:], pgt[:])

            m1 = hp.tile([P, kt2, n_graphs], f32, tag="m1")
            nc.vector.memset(m1[:], 0.0)
            for k in range(kt2):
                mwid = min(P, s1tot - k * P)
                pm = hpp.tile([P, n_graphs], f32, tag="pm1")
                nc.tensor.matmul(pm[:mwid, :], lhsT=wm1_sb[:, k * P:k * P + mwid],
                                 rhs=gT[:], start=True, stop=True)
                nc.scalar.activation(m1[:mwid, k, :], pm[:mwid, :], ACT.Relu)
            pm2 = hpp.tile([s2tot, n_graphs], f32, tag="pm2")
            for k in range(kt2):
                nc.tensor.matmul(pm2[:], lhsT=wm2_sb[:, k, :], rhs=m1[:, k, :],
                                 start=(k == 0), stop=(k == kt2 - 1))
            m2 = hp.tile([P, n_graphs], f32, tag="m2")
            nc.vector.memset(m2[:], 0.0)
            if rlo > 0:
                nc.vector.tensor_copy(m2[0:rlo, :], pm2[0:rlo, :])
            nc.scalar.activation(m2[rlo:rhi, :], pm2[rlo:rhi, :], ACT.Relu)
            if rhi < s2tot:
                nc.vector.tensor_copy(m2[rhi:s2tot, :], pm2[rhi:s2tot, :])
            pm3 = hpp.tile([s3tot, n_graphs], f32, tag="pm3")
            nc.tensor.matmul(pm3[:], lhsT=wm3_sb[:], rhs=m2[:], start=True,
                             stop=True)
            osb = hp.tile([s3tot, n_graphs], f32, tag="osb")
            nc.vector.tensor_copy(osb[:], pm3[:])
            nc.sync.dma_start(out_d.ap(), osb[:])

    nc.compile()
    return nc, in_names


# ------------------------------------------------------------------ driver --

def _host_inputs(x, edge_index, batch, params, meta, cfgs, cores):
    """Per-core input tensors for the program."""
    import ml_dtypes

    x = np.asarray(x, np.float32)
    batch = np.asarray(batch)
    n_nodes = x.shape[0]
    n_graphs = 64
    npc = n_nodes // cores
    ndt = meta["ndt"]

    g1, g2, g3 = params["gat1"], params["gat2"], params["gat3"]
    fin1 = x.shape[1]
    w1 = _aug_weights(g1, 8, fin1, 128)
    w2 = _aug_weights(g2, 4, 1024, 128)
    w3 = _aug_weights(g3, 1, 512, 128)
    for g in (g1, g2, g3):
        assert not np.any(np.asarray(g["b"])), "nonzero GAT bias unsupported"
    W1m, W2m, W3m, *_ = _mlp_weights(params["heads"])

    cnts = np.bincount(batch, minlength=n_graphs).astype(np.float64)
    cnts = np.maximum(cnts, 1.0)
    spool = np.zeros((cores, ndt, P, n_graphs), np.float32)
    inv = 1.0 / cnts[batch]
    for c in range(cores):
        for d in range(ndt):
            pn = min(P, npc - d * P)
            n0 = c * npc + d * P
            spool[c, d, np.arange(pn), batch[n0:n0 + pn]] = inv[n0:n0 + pn]

    kt1 = cfgs[0]["kt"]
    nft = math.ceil(n_nodes / P)
    xT = np.zeros((kt1, nft, P, P), np.float32)
    for k in range(kt1):
        for d in range(nft):
            blk = x[d * P:(d + 1) * P, k * P:(k + 1) * P]
            xT[k, d, :blk.shape[1], :blk.shape[0]] = blk.T

    iota = np.tile(np.arange(P, dtype=np.float32), (P, 1))

    in_maps = []
    for c in range(cores):
        in_maps.append({
            "xT": xT,
            "w1": w1, "w2": w2, "w3": w3,
            "srcw": _wrap_idx(meta["srcw"][c]),
            "dstw": _wrap_idx(meta["dstw"][c]),
            "dstloc": np.ascontiguousarray(
                meta["dstloc"][c].reshape(meta["nt"], P).T.astype(np.float32)),
            "iota": iota.astype(ml_dtypes.bfloat16),
            "spool": np.ascontiguousarray(spool[c]),
            "wm1": W1m, "wm2": W2m, "wm3": W3m,
        })
    return in_maps


def _run(x, edge_index, batch, params, cores=8, trace=False):
    from concourse.bass_utils import run_bass_kernel_spmd

    x = np.asarray(x, np.float32)
    edge_index = np.asarray(edge_index)
    batch = np.asarray(batch)
    n_nodes = x.shape[0]
    n_graphs = 64

    meta = _preprocess(edge_index, n_nodes, cores)
    fin1 = x.shape[1]
    cfgs = [
        dict(h=8, fout=1024, kt=math.ceil(fin1 / P)),
        dict(h=4, fout=512, kt=8),
        dict(h=1, fout=128, kt=4),
    ]
    _, _, _, s1tot, s2tot, s3tot, relu_rows = _mlp_weights(params["heads"])
    nc, _ = _build_program(meta, cores, cfgs, n_graphs,
                           (s1tot, s2tot, s3tot, relu_rows))
    in_maps = _host_inputs(x, edge_index, batch, params, meta, cfgs, cores)
    res = run_bass_kernel_spmd(nc, in_maps, core_ids=list(range(cores)),
                               trace=trace)
    out = res.results[0]["out_T"]  # [18, 64]
    return np.ascontiguousarray(out.T.astype(np.float32)), res


def kernel(x, edge_index, batch, params):
    out, _ = _run(x, edge_index, batch, params)
    return out


# revision 43
# speedup vs baseline: 66.3450x; 1.0871x over previous
"""ClinicalGAT Trainium2 kernel: 3 GAT layers + mean-pool + slot-head MLPs.

Strategy (8-core SPMD, graph-parallel over destination nodes):
  - Nodes are partitioned contiguously across the 8 cores (1250 each).
  - Per layer: each core computes h_aug = h_prev @ [W | v_src | v_dst] for its
    node shard (PE, node-major via PE-transposed lhsT tiles), writes a bf16
    feature table shard + f32 attention-logit ("al") shard to HBM, and the
    shards are AllGather'd (chunked, overlapping with the matmuls).
  - Attention: edges are pre-sorted by destination host-side and padded so each
    128-edge tile maps to exactly one 128-destination tile.  Per edge chunk:
    dma_gather pulls source-node feature rows (bf16) and src/dst al rows (f32);
    softmax weights w = exp(leaky_relu(al_src+al_dst)) (no max-subtraction --
    logits are bounded); a selection matrix A[e, j] = (dstloc[e] == j) is built
    on DVE, scaled per-head by w, and PE matmuls A_w.T @ gathered_h accumulate
    the weighted message sums per destination tile in PSUM.  The softmax
    denominator comes from an extra matmul w.T @ A into the same PSUM group.
  - Epilogue divides by the denominator, adds bias, applies ELU.
  - Mean-pool is a matmul against a host-built (1/count-scaled) selection
    matrix, AllReduce across cores, and the slot-head MLPs are three small
    block-diagonal matmuls.
"""

import math
import numpy as np

P = 128
CH = 8  # edge tiles per gather chunk


# ---------------------------------------------------------------- host side --

def _superchunks(ndt, npc):
    """Groups of up to 4 node tiles; returns [(d0, d1, n_rows)]."""
    out = []
    d = 0
    while d < ndt:
        d1 = min(d + 4, ndt)
        rows = min(d1 * P, npc) - d * P
        out.append((d, d1, rows))
        d = d1
    return out


def _remap(node, npc, cores, scs):
    """Map global node id -> row in the chunk-allgathered table layout."""
    c = node // npc
    l = node % npc
    base = 0
    for (d0, d1, rows) in scs:
        lo, hi = d0 * P, d0 * P + rows
        if lo <= l < hi:
            return base + c * rows + (l - lo)
        base += cores * rows
    raise AssertionError


def _preprocess(edge_index, n_nodes, cores):
    npc = n_nodes // cores
    ndt = math.ceil(npc / P)
    scs = _superchunks(ndt, npc)

    loop = np.arange(n_nodes, dtype=np.int64)
    src = np.concatenate([edge_index[0].astype(np.int64), loop])
    dst = np.concatenate([edge_index[1].astype(np.int64), loop])
    order = np.argsort(dst, kind="stable")
    src, dst = src[order], dst[order]

    remap_tab = np.arange(n_nodes, dtype=np.int64)  # table rows = node order

    # split per (core, dst-tile)
    per = [[None] * ndt for _ in range(cores)]
    for c in range(cores):
        lo, hi = c * npc, (c + 1) * npc
        m = (dst >= lo) & (dst < hi)
        s_c, d_c = src[m], dst[m] - lo
        for t in range(ndt):
            mt = (d_c >= t * P) & (d_c < min((t + 1) * P, npc))
            per[c][t] = (s_c[mt], d_c[mt] - t * P)

    slot_tiles = [max(math.ceil(max(len(per[c][t][0]), 1) / P) for c in range(cores))
                  for t in range(ndt)]
    nt = sum(slot_tiles)
    nt_pad = math.ceil(nt / CH) * CH
    slot_tiles[-1] += nt_pad - nt  # trailing pad tiles extend the last dst tile
    nt = nt_pad
    ne = nt * P

    tile2dst, first, last = [], [], []
    for t in range(ndt):
        for k in range(slot_tiles[t]):
            tile2dst.append(t)
            first.append(k == 0)
            last.append(k == slot_tiles[t] - 1)

    srcw = np.zeros((cores, ne), dtype=np.int64)
    dstw = np.zeros((cores, ne), dtype=np.int64)
    dstloc = np.full((cores, ne), -1.0, dtype=np.float64)
    for c in range(cores):
        pos = 0
        for t in range(ndt):
            s_t, dl_t = per[c][t]
            k = len(s_t)
            srcw[c, pos:pos + k] = remap_tab[s_t]
            dstw[c, pos:pos + k] = remap_tab[c * npc + t * P + dl_t]
            dstloc[c, pos:pos + k] = dl_t
            pos += slot_tiles[t] * P
    return dict(npc=npc, ndt=ndt, scs=scs, nt=nt, ne=ne,
                tile2dst=tile2dst, first=first, last=last,
                srcw=srcw, dstw=dstw, dstloc=dstloc)


def _wrap_idx(idx):
    """[NE] -> wrapped int16 [128, NE/16] (16-partition wrap, replicated x8)."""
    ne = idx.shape[0]
    assert ne % 16 == 0
    a = idx.reshape(ne // 16, 16).T.astype(np.int16)
    return np.ascontiguousarray(np.tile(a, (8, 1)))


def _aug_weights(g, heads, din, dout):
    W = np.asarray(g["W"], np.float32)
    a_src = np.asarray(g["a_src"], np.float32)
    a_dst = np.asarray(g["a_dst"], np.float32)
    Wr = W.reshape(din, heads, dout)
    vsrc = np.einsum("fhd,hd->fh", Wr, a_src).astype(np.float32)
    vdst = np.einsum("fhd,hd->fh", Wr, a_dst).astype(np.float32)
    Waug = np.concatenate([W, vsrc, vdst], axis=1)
    kt = math.ceil(din / P)
    pad = np.zeros((kt * P - din, Waug.shape[1]), np.float32)
    Waug = np.concatenate([Waug, pad], 0)
    return np.ascontiguousarray(Waug.reshape(kt, P, Waug.shape[1]))


def _mlp_weights(heads_params):
    """Three combined matrices for the 8 slot-head MLPs.

    m1 = relu(g @ W1) [64, s1tot]; m2 = g1 @ W2 (relu on ternary slice);
    out = m2' @ W3.  W2/W3 are block-diagonal (identity passthrough for
    2-layer heads in W3).
    """
    sizes1 = [np.asarray(l[0][0]).shape[1] for l in heads_params]  # first-layer widths
    s1tot = sum(sizes1)
    W1 = np.zeros((P, s1tot), np.float32)
    col = 0
    for l, s in zip(heads_params, sizes1):
        W1[:np.asarray(l[0][0]).shape[0], col:col + s] = np.asarray(l[0][0], np.float32)
        col += s

    # second stage: per head either final (2-layer head) or middle (3-layer).
    # 3-layer heads are placed FIRST along the m2 feature axis so the relu
    # slice starts at partition 0 (partition offsets must be multiples of 32).
    sizes2 = [np.asarray(l[1][0]).shape[1] for l in heads_params]
    s2tot = sum(sizes2)
    order = [i for i, l in enumerate(heads_params) if len(l) == 3] + \
            [i for i, l in enumerate(heads_params) if len(l) != 3]
    row_off = np.cumsum([0] + sizes1)  # into m1 (head order)
    c2_off = {}
    c = 0
    for i in order:
        c2_off[i] = c
        c += sizes2[i]
    W2 = np.zeros((s1tot, s2tot), np.float32)
    relu_cols = []
    for i, l in enumerate(heads_params):
        W2[row_off[i]:row_off[i] + sizes1[i],
           c2_off[i]:c2_off[i] + sizes2[i]] = np.asarray(l[1][0], np.float32)
        if len(l) == 3:
            relu_cols.append((c2_off[i], c2_off[i] + sizes2[i]))

    sizes3 = [(np.asarray(l[2][0]).shape[1] if len(l) == 3 else s2)
              for l, s2 in zip(heads_params, sizes2)]
    s3tot = sum(sizes3)
    c3_off = np.cumsum([0] + sizes3)  # output stays in head order
    W3 = np.zeros((s2tot, s3tot), np.float32)
    for i, l in enumerate(heads_params):
        r, c = c2_off[i], c3_off[i]
        if len(l) == 3:
            W3[r:r + sizes2[i], c:c + sizes3[i]] = np.asarray(l[2][0], np.float32)
        else:
            W3[r:r + sizes2[i], c:c + sizes3[i]] = np.eye(sizes2[i], dtype=np.float32)

    kt2 = math.ceil(s1tot / P)
    W2p = np.zeros((kt2, P, s2tot), np.float32)
    for k in range(kt2):
        W2p[k, :min(P, s1tot - k * P), :] = W2[k * P:(k + 1) * P, :]
    W3p = np.zeros((P, s3tot), np.float32)
    W3p[:s2tot] = W3
    # relu rows of the m2 intermediate (feature-major partitions)
    assert relu_cols, "expected at least one 3-layer head"
    rlo = min(a for a, _ in relu_cols)
    rhi = max(b for _, b in relu_cols)
    for l in heads_params:
        for _, b, _ in l:
            assert not np.any(np.asarray(b)), "nonzero MLP bias unsupported"
    return W1, W2p, W3p, s1tot, s2tot, s3tot, (rlo, rhi)


# ------------------------------------------------------------- program side --

def _build_program(meta, cores, cfgs, n_graphs, mlp_shapes):
    """Emit the full Tile program; returns (nc, input tensor names)."""
    from concourse import bass, mybir, tile
    from concourse import bacc
    from concourse.masks import make_identity
    from contextlib import ExitStack

    dt = mybir.dt
    f32, bf16, i16 = dt.float32, dt.bfloat16, dt.int16
    OP = mybir.AluOpType
    ACT = mybir.ActivationFunctionType

    npc, ndt, nt, ne = meta["npc"], meta["ndt"], meta["nt"], meta["ne"]
    tile2dst, tfirst, tlast = meta["tile2dst"], meta["first"], meta["last"]
    s1tot, s2tot, s3tot, (rlo, rhi) = mlp_shapes
    rows_l = [1152, 640, 256]  # bf16 table row: fout | f32-bitcast al | pad

    nc = bacc.Bacc("TRN2", target_bir_lowering=False, debug=False,
                   num_devices=cores)
    groups = [list(range(cores))]

    # ---- external inputs
    kt1 = cfgs[0]["kt"]
    n_total = npc * cores
    nft = math.ceil(n_total / P)  # layer-0 stage is replicated over all nodes
    xT_d = nc.dram_tensor("xT", [kt1, nft, P, P], f32, kind="ExternalInput")
    w_d = [nc.dram_tensor(f"w{i+1}", [c["kt"], P, c["fout"] + 2 * c["h"]], f32,
                          kind="ExternalInput") for i, c in enumerate(cfgs)]
    srcw_d = nc.dram_tensor("srcw", [P, ne // 16], i16, kind="ExternalInput")
    dstw_d = nc.dram_tensor("dstw", [P, ne // 16], i16, kind="ExternalInput")
    dstloc_d = nc.dram_tensor("dstloc", [P, nt], f32, kind="ExternalInput")
    iota_d = nc.dram_tensor("iota", [P, P], bf16, kind="ExternalInput")
    spool_d = nc.dram_tensor("spool", [ndt, P, n_graphs], f32, kind="ExternalInput")
    wm1_d = nc.dram_tensor("wm1", [P, s1tot], f32, kind="ExternalInput")
    kt2 = math.ceil(s1tot / P)
    wm2_d = nc.dram_tensor("wm2", [kt2, P, s2tot], f32, kind="ExternalInput")
    wm3_d = nc.dram_tensor("wm3", [P, s3tot], f32, kind="ExternalInput")
    out_d = nc.dram_tensor("out_T", [s3tot, n_graphs], f32, kind="ExternalOutput")

    in_names = ["xT", "w1", "w2", "w3", "srcw", "dstw", "dstloc", "iota",
                "spool", "wm1", "wm2", "wm3"]

    with tile.TileContext(nc) as tc, ExitStack() as ctx:
        dram = ctx.enter_context(tc.tile_pool(name="dram", bufs=1, space="DRAM"))
        tables = [dram.tile([n_total, rows_l[i]], bf16, tag=f"tab{i}",
                            name=f"tab{i}")
                  for i, c in enumerate(cfgs)]
        shards = [None] + [dram.tile([npc, rows_l[i]], bf16, tag=f"shard{i}",
                                     name=f"shard{i}")
                           for i, c in list(enumerate(cfgs))[1:]]
        g_in = dram.tile([n_graphs, P], f32, tag="g_in")
        g_out = dram.tile([n_graphs, P], f32, tag="g_out")

        cpool = ctx.enter_context(tc.tile_pool(name="const", bufs=1))
        ident = cpool.tile([P, P], f32, tag="ident")
        make_identity(nc, ident[:])
        iota_sb = cpool.tile([P, P], bf16, tag="iota")
        nc.sync.dma_start(iota_sb[:], iota_d.ap())
        srcw = cpool.tile([P, ne // 16], i16, tag="srcw")
        nc.sync.dma_start(srcw[:], srcw_d.ap())
        dstw = cpool.tile([P, ne // 16], i16, tag="dstw")
        nc.sync.dma_start(dstw[:], dstw_d.ap())
        dstloc = cpool.tile([P, nt], f32, tag="dstloc")
        nc.sync.dma_start(dstloc[:], dstloc_d.ap())
        spool_sb = cpool.tile([P, ndt, n_graphs], f32, tag="spool")
        nc.sync.dma_start(spool_sb[:], spool_d.ap().rearrange("d p g -> p d g"))
        wm1_sb = cpool.tile([P, s1tot], f32, tag="wm1")
        nc.sync.dma_start(wm1_sb[:], wm1_d.ap())
        wm2_sb = cpool.tile([P, kt2, s2tot], f32, tag="wm2")
        nc.sync.dma_start(wm2_sb[:], wm2_d.ap().rearrange("k p n -> p k n"))
        wm3_sb = cpool.tile([P, s3tot], f32, tag="wm3")
        nc.sync.dma_start(wm3_sb[:], wm3_d.ap())

        hpool = ctx.enter_context(tc.tile_pool(name="hsb", bufs=1))
        hsb = [hpool.tile([P, ndt, c["fout"]], f32, tag=f"hsb{i}", name=f"hsb{i}")
               for i, c in enumerate(cfgs)]

        sctx = ExitStack()
        sp = sctx.enter_context(tc.tile_pool(name="stg", bufs=2))
        swp = sctx.enter_context(tc.tile_pool(name="stgw", bufs=1))
        spp = sctx.enter_context(tc.tile_pool(name="stgp", bufs=2, space="PSUM"))
        for L, c in enumerate(cfgs):
            h, fout, ktl = c["h"], c["fout"], c["kt"]
            naug = fout + 2 * h
            if True:
                # ---------- stage: h_aug matmuls + shard writes + allgather
                wsb_l = swp.tile([P, ktl, naug], f32, tag="wsb_l",
                                 name=f"wsb_l{L}")
                nc.sync.dma_start(wsb_l[:], w_d[L].ap().rearrange("k p n -> p k n"))
                # walrus requires fp32r matmul operands to be explicitly
                # rounded by their producing instruction
                wsb_r = swp.tile([P, ktl, naug], dt.float32r, tag="wsb_r",
                                 name=f"wsb_r{L}")
                nc.vector.tensor_copy(wsb_r[:], wsb_l[:])
                # L0 is replicated over all node tiles (no collective); L>0
                # computes its own shard then one AllGather.
                row = rows_l[L]
                dest = tables[0] if L == 0 else shards[L]
                n_rows = n_total if L == 0 else npc
                n_dt = nft if L == 0 else ndt
                for d in range(n_dt):
                    dn = min(n_rows - d * P, P)
                    lt = sp.tile([P, ktl, P], dt.float32r, tag="lhs")
                    if L == 0:
                        lt0 = sp.tile([P, ktl, P], f32, tag="lhs0")
                        nc.sync.dma_start(
                            lt0[:], xT_d.ap()[:, d, :, :].rearrange("k p q -> p k q"))
                        nc.vector.tensor_copy(lt[:], lt0[:])
                    else:
                        for k in range(ktl):
                            pt = spp.tile([P, P], f32, tag="ptr")
                            nc.tensor.transpose(
                                out=pt[:], identity=ident[:],
                                in_=hsb[L - 1][:, d, k * P:(k + 1) * P])
                            nc.vector.tensor_copy(lt[:, k, :], pt[:])
                    lhs = [lt[:, k, :] for k in range(ktl)]
                    nsplits = []
                    c0 = 0
                    while c0 < naug:
                        c1 = min(c0 + 512, naug)
                        if c0 < fout < c1:
                            c1 = fout
                        nsplits.append((c0, c1))
                        c0 = c1
                    stf = sp.tile([P, row], bf16, tag="stb")
                    for (c0, c1) in nsplits:
                        ps = spp.tile([P, 512], f32, tag="pstage")
                        for k in range(ktl):
                            nc.tensor.matmul(
                                ps[:, :c1 - c0], lhsT=lhs[k],
                                rhs=wsb_r[:, k, c0:c1],
                                start=(k == 0), stop=(k == ktl - 1))
                        if c1 <= fout:
                            nc.scalar.activation(stf[:, c0:c1], ps[:, :c1 - c0],
                                                 ACT.Copy)
                        else:
                            # al cols: f32 bitcast into the bf16 row + zero pad
                            nc.vector.memset(stf[:, fout + 4 * h:row], 0)
                            nc.vector.tensor_copy(
                                stf[:, fout:fout + 4 * h].bitcast(f32),
                                ps[:, :2 * h])
                    nc.sync.dma_start(dest[d * P:d * P + dn, :], stf[:dn, :])
                if L > 0:
                    nc.gpsimd.collective_compute(
                        "AllGather", mybir.AluOpType.bypass, groups,
                        ins=[shards[L][:, :].opt()],
                        outs=[tables[L][:, :].opt()])

            # ---------- attention
            with tc.tile_pool(name=f"att{L}", bufs=3) as ap, \
                 tc.tile_pool(name=f"attw{L}", bufs=2) as wp, \
                 tc.tile_pool(name=f"aw{L}", bufs=3) as awp, \
                 tc.tile_pool(name=f"attp{L}", bufs=1, space="PSUM") as pp, \
                 tc.tile_pool(name=f"attps{L}", bufs=1, space="PSUM") as pps, \
                 tc.tile_pool(name=f"attps2{L}", bufs=1, space="PSUM") as pps1:
                npacks = math.ceil(h * P / 512)
                packs = None
                den = None
                row = rows_l[L]
                chl = CH  # 1024 descs/call, proven safe on HW
                nchunks = nt // chl
                for ch in range(nchunks):
                    i0 = ch * chl * 8  # wrapped idx col offset
                    g = ap.tile([P, chl, row], bf16, tag="g")
                    nc.gpsimd.dma_gather(
                        g[:], tables[L][:, :], srcw[:, i0:i0 + chl * 8],
                        num_idxs=chl * P, num_idxs_reg=chl * P, elem_size=row)
                    adg = ap.tile([P, chl, P], bf16, tag="adg")
                    nc.gpsimd.dma_gather(
                        adg[:], tables[L][:, fout:fout + P],
                        dstw[:, i0:i0 + chl * 8],
                        num_idxs=chl * P, num_idxs_reg=chl * P, elem_size=P,
                        elem_step=row)
                    # al sections are f32 bitcast inside the bf16 rows
                    wf = wp.tile([P, chl, h], f32, tag="wf")
                    nc.vector.tensor_tensor(
                        out=wf[:],
                        in0=g[:, :, fout:fout + 4 * h].bitcast(f32)[:, :, 0:h],
                        in1=adg[:, :, 0:4 * h].bitcast(f32)[:, :, h:2 * h],
                        op=OP.add)
                    lrt = wp.tile([P, chl, h], f32, tag="lrt")
                    nc.vector.tensor_scalar(
                        out=lrt[:], in0=wf[:], scalar1=0.2, scalar2=None,
                        op0=OP.mult)
                    nc.vector.tensor_tensor(
                        out=wf[:], in0=wf[:], in1=lrt[:], op=OP.max)
                    nc.scalar.activation(wf[:], wf[:], ACT.Exp)
                    wb = wp.tile([P, chl, h], bf16, tag="wb")
                    nc.vector.tensor_copy(wb[:], wf[:])
                    for t in range(chl):
                        gt = ch * chl + t
                        Ab = awp.tile([P, P], bf16, tag="Ab", name="Ab")
                        nc.vector.tensor_scalar(
                            out=Ab[:], in0=iota_sb[:],
                            scalar1=dstloc[:, gt:gt + 1], scalar2=None,
                            op0=OP.is_equal)
                        Aw = awp.tile([P, h, P], bf16, tag="Aw")
                        for hh in range(h):
                            nc.vector.tensor_scalar(
                                out=Aw[:, hh, :], in0=iota_sb[:],
                                scalar1=dstloc[:, gt:gt + 1],
                                scalar2=wf[:, t, hh:hh + 1],
                                op0=OP.is_equal, op1=OP.mult)
                        dtile = tile2dst[gt]
                        if tfirst[gt]:
                            packs = [pp.tile([P, 512], f32, tag=f"pk{i}", name=f"pk{i}")
                                     for i in range(npacks)]
                            den = pps.tile([h, P], f32, tag="den")
                        for hh in range(h):
                            pk = packs[(hh * P) // 512]
                            off = (hh * P) % 512
                            # start=True resets has_written for the whole
                            # PSUM tile -- only the first write to each pack
                            # may set it.
                            nc.tensor.matmul(
                                pk[:, off:off + P], lhsT=Aw[:, hh, :],
                                rhs=g[:, t, hh * P:(hh + 1) * P],
                                start=(tfirst[gt] and off == 0),
                                stop=tlast[gt],
                                skip_group_check=True)
                        nc.tensor.matmul(
                            den[:, :], lhsT=wb[:, t, :], rhs=Ab[:],
                            start=tfirst[gt], stop=tlast[gt],
                            skip_group_check=True)
                        if tlast[gt]:
                            # epilogue for dst tile `dtile`
                            dsb = wp.tile([h, P], f32, tag="dsb")
                            nc.vector.tensor_copy(dsb[:], den[:, :])
                            dtp = pps1.tile([P, h], f32, tag="dtp")
                            nc.tensor.transpose(out=dtp[:], in_=dsb[:],
                                                identity=ident[:h, :h])
                            r = wp.tile([P, h], f32, tag="rcp")
                            nc.vector.tensor_scalar(
                                out=r[:], in0=dtp[:], scalar1=1e-16,
                                scalar2=None, op0=OP.add)
                            nc.vector.reciprocal(r[:], r[:])
                            hv = hsb[L][:, dtile, :]
                            for hh in range(h):
                                pk = packs[(hh * P) // 512]
                                off = (hh * P) % 512
                                nc.scalar.activation(
                                    hv[:, hh * P:(hh + 1) * P],
                                    pk[:, off:off + P], ACT.Copy,
                                    scale=r[:, hh:hh + 1])
                            # elu(v) = max(v, min(exp(v), 1) - 1)
                            tmp = wp.tile([P, fout], f32, tag="elu")
                            nc.scalar.activation(tmp[:], hv, ACT.Exp)
                            nc.vector.tensor_scalar(
                                out=tmp[:], in0=tmp[:], scalar1=1.0,
                                scalar2=-1.0, op0=OP.min, op1=OP.add)
                            nc.vector.tensor_tensor(
                                out=hv, in0=hv, in1=tmp[:], op=OP.max)

        # ---------- mean pool + heads
        sctx.close()  # release stage pools; head phase needs the PSUM banks
        with tc.tile_pool(name="head", bufs=1) as hp, \
             tc.tile_pool(name="headp", bufs=1, space="PSUM") as hpp:
            psg = hpp.tile([n_graphs, P], f32, tag="psg")
            for d in range(ndt):
                nc.tensor.matmul(psg[:], lhsT=spool_sb[:, d, :],
                                 rhs=hsb[2][:, d, :],
                                 start=(d == 0), stop=(d == ndt - 1))
            gsb = hp.tile([n_graphs, P], f32, tag="gsb")
            nc.vector.tensor_copy(gsb[:], psg[:])
            nc.sync.dma_start(g_in[:, :], gsb[:])
            nc.gpsimd.collective_compute(
                "AllReduce", mybir.AluOpType.add, groups,
                ins=[g_in[:, :].opt()], outs=[g_out[:, :].opt()])
            gfull = hp.tile([n_graphs, P], f32, tag="gfull")
            nc.sync.dma_start(gfull[:], g_out[:, :])
            pgt = hpp.tile([P, n_graphs], f32, tag="pgt")
            nc.tensor.transpose(out=pgt[:], in_=gfull[:],
                                identity=ident[:n_graphs, :n_graphs])
            gT = hp.tile([P, n_graphs], f32, tag="gT")
            nc.vector.tensor_copy(gT[:], pgt[:])

            m1 = hp.tile([P, kt2, n_graphs], f32, tag="m1")
            nc.vector.memset(m1[:], 0.0)
            for k in range(kt2):
                mwid = min(P, s1tot - k * P)
                pm = hpp.tile([P, n_graphs], f32, tag="pm1")
                nc.tensor.matmul(pm[:mwid, :], lhsT=wm1_sb[:, k * P:k * P + mwid],
                                 rhs=gT[:], start=True, stop=True)
                nc.scalar.activation(m1[:mwid, k, :], pm[:mwid, :], ACT.Relu)
            pm2 = hpp.tile([s2tot, n_graphs], f32, tag="pm2")
            for k in range(kt2):
                nc.tensor.matmul(pm2[:], lhsT=wm2_sb[:, k, :], rhs=m1[:, k, :],
                                 start=(k == 0), stop=(k == kt2 - 1))
            m2 = hp.tile([P, n_graphs], f32, tag="m2")
            nc.vector.memset(m2[:], 0.0)
            if rlo > 0:
                nc.vector.tensor_copy(m2[0:rlo, :], pm2[0:rlo, :])
            nc.scalar.activation(m2[rlo:rhi, :], pm2[rlo:rhi, :], ACT.Relu)
            if rhi < s2tot:
                nc.vector.tensor_copy(m2[rhi:s2tot, :], pm2[rhi:s2tot, :])
            pm3 = hpp.tile([s3tot, n_graphs], f32, tag="pm3")
            nc.tensor.matmul(pm3[:], lhsT=wm3_sb[:], rhs=m2[:], start=True,
                             stop=True)
            osb = hp.tile([s3tot, n_graphs], f32, tag="osb")
            nc.vector.tensor_copy(osb[:], pm3[:])
            nc.sync.dma_start(out_d.ap(), osb[:])

    nc.compile()
    return nc, in_names


# ------------------------------------------------------------------ driver --

def _host_inputs(x, edge_index, batch, params, meta, cfgs, cores):
    """Per-core input tensors for the program."""
    import ml_dtypes

    x = np.asarray(x, np.float32)
    batch = np.asarray(batch)
    n_nodes = x.shape[0]
    n_graphs = 64
    npc = n_nodes // cores
    ndt = meta["ndt"]

    g1, g2, g3 = params["gat1"], params["gat2"], params["gat3"]
    fin1 = x.shape[1]
    w1 = _aug_weights(g1, 8, fin1, 128)
    w2 = _aug_weights(g2, 4, 1024, 128)
    w3 = _aug_weights(g3, 1, 512, 128)
    for g in (g1, g2, g3):
        assert not np.any(np.asarray(g["b"])), "nonzero GAT bias unsupported"
    W1m, W2m, W3m, *_ = _mlp_weights(params["heads"])

    cnts = np.bincount(batch, minlength=n_graphs).astype(np.float64)
    cnts = np.maximum(cnts, 1.0)
    spool = np.zeros((cores, ndt, P, n_graphs), np.float32)
    inv = 1.0 / cnts[batch]
    for c in range(cores):
        for d in range(ndt):
            pn = min(P, npc - d * P)
            n0 = c * npc + d * P
            spool[c, d, np.arange(pn), batch[n0:n0 + pn]] = inv[n0:n0 + pn]

    kt1 = cfgs[0]["kt"]
    nft = math.ceil(n_nodes / P)
    xT = np.zeros((kt1, nft, P, P), np.float32)
    for k in range(kt1):
        for d in range(nft):
            blk = x[d * P:(d + 1) * P, k * P:(k + 1) * P]
            xT[k, d, :blk.shape[1], :blk.shape[0]] = blk.T

    iota = np.tile(np.arange(P, dtype=np.float32), (P, 1))

    in_maps = []
    for c in range(cores):
        in_maps.append({
            "xT": xT,
            "w1": w1, "w2": w2, "w3": w3,
            "srcw": _wrap_idx(meta["srcw"][c]),
            "dstw": _wrap_idx(meta["dstw"][c]),
            "dstloc": np.ascontiguousarray(
                meta["dstloc"][c].reshape(meta["nt"], P).T.astype(np.float32)),
            "iota": iota.astype(ml_dtypes.bfloat16),
            "spool": np.ascontiguousarray(spool[c]),
            "wm1": W1m, "wm2": W2m, "wm3": W3m,
        })
    return in_maps


def _run(x, edge_index, batch, params, cores=8, trace=False):
    from concourse.bass_utils import run_bass_kernel_spmd

    x = np.asarray(x, np.float32)
    edge_index = np.asarray(edge_index)
    batch = np.asarray(batch)
    n_nodes = x.shape[0]
    n_graphs = 64

    meta = _preprocess(edge_index, n_nodes, cores)
    fin1 = x.shape[1]
    cfgs = [
        dict(h=8, fout=1024, kt=math.ceil(fin1 / P)),
        dict(h=4, fout=512, kt=8),
        dict(h=1, fout=128, kt=4),
    ]
    _, _, _, s1tot, s2tot, s3tot, relu_rows = _mlp_weights(params["heads"])
    nc, _ = _build_program(meta, cores, cfgs, n_graphs,
                           (s1tot, s2tot, s3tot, relu_rows))
    in_maps = _host_inputs(x, edge_index, batch, params, meta, cfgs, cores)
    res = run_bass_kernel_spmd(nc, in_maps, core_ids=list(range(cores)),
                               trace=trace)
    out = res.results[0]["out_T"]  # [18, 64]
    return np.ascontiguousarray(out.T.astype(np.float32)), res


def kernel(x, edge_index, batch, params):
    out, _ = _run(x, edge_index, batch, params)
    return out


# revision 47
# speedup vs baseline: 66.3627x; 1.0003x over previous
"""ClinicalGAT Trainium2 kernel: 3 GAT layers + mean-pool + slot-head MLPs.

Strategy (8-core SPMD, graph-parallel over destination nodes):
  - Nodes are partitioned contiguously across the 8 cores (1250 each).
  - Per layer: each core computes h_aug = h_prev @ [W | v_src | v_dst] for its
    node shard (PE, node-major via PE-transposed lhsT tiles), writes a bf16
    feature table shard + f32 attention-logit ("al") shard to HBM, and the
    shards are AllGather'd (chunked, overlapping with the matmuls).
  - Attention: edges are pre-sorted by destination host-side and padded so each
    128-edge tile maps to exactly one 128-destination tile.  Per edge chunk:
    dma_gather pulls source-node feature rows (bf16) and src/dst al rows (f32);
    softmax weights w = exp(leaky_relu(al_src+al_dst)) (no max-subtraction --
    logits are bounded); a selection matrix A[e, j] = (dstloc[e] == j) is built
    on DVE, scaled per-head by w, and PE matmuls A_w.T @ gathered_h accumulate
    the weighted message sums per destination tile in PSUM.  The softmax
    denominator comes from an extra matmul w.T @ A into the same PSUM group.
  - Epilogue divides by the denominator, adds bias, applies ELU.
  - Mean-pool is a matmul against a host-built (1/count-scaled) selection
    matrix, AllReduce across cores, and the slot-head MLPs are three small
    block-diagonal matmuls.
"""

import math
import numpy as np

P = 128
CH = 8  # edge tiles per gather chunk


# ---------------------------------------------------------------- host side --

def _superchunks(ndt, npc):
    """Groups of up to 4 node tiles; returns [(d0, d1, n_rows)]."""
    out = []
    d = 0
    while d < ndt:
        d1 = min(d + 4, ndt)
        rows = min(d1 * P, npc) - d * P
        out.append((d, d1, rows))
        d = d1
    return out


def _remap(node, npc, cores, scs):
    """Map global node id -> row in the chunk-allgathered table layout."""
    c = node // npc
    l = node % npc
    base = 0
    for (d0, d1, rows) in scs:
        lo, hi = d0 * P, d0 * P + rows
        if lo <= l < hi:
            return base + c * rows + (l - lo)
        base += cores * rows
    raise AssertionError


def _preprocess(edge_index, n_nodes, cores):
    npc = n_nodes // cores
    ndt = math.ceil(npc / P)
    scs = _superchunks(ndt, npc)

    loop = np.arange(n_nodes, dtype=np.int64)
    src = np.concatenate([edge_index[0].astype(np.int64), loop])
    dst = np.concatenate([edge_index[1].astype(np.int64), loop])
    order = np.argsort(dst, kind="stable")
    src, dst = src[order], dst[order]

    remap_tab = np.arange(n_nodes, dtype=np.int64)  # table rows = node order

    # split per (core, dst-tile)
    per = [[None] * ndt for _ in range(cores)]
    for c in range(cores):
        lo, hi = c * npc, (c + 1) * npc
        m = (dst >= lo) & (dst < hi)
        s_c, d_c = src[m], dst[m] - lo
        for t in range(ndt):
            mt = (d_c >= t * P) & (d_c < min((t + 1) * P, npc))
            per[c][t] = (s_c[mt], d_c[mt] - t * P)

    slot_tiles = [max(math.ceil(max(len(per[c][t][0]), 1) / P) for c in range(cores))
                  for t in range(ndt)]
    nt = sum(slot_tiles)
    nt_pad = math.ceil(nt / CH) * CH
    slot_tiles[-1] += nt_pad - nt  # trailing pad tiles extend the last dst tile
    nt = nt_pad
    ne = nt * P

    tile2dst, first, last = [], [], []
    for t in range(ndt):
        for k in range(slot_tiles[t]):
            tile2dst.append(t)
            first.append(k == 0)
            last.append(k == slot_tiles[t] - 1)

    srcw = np.zeros((cores, ne), dtype=np.int64)
    dstw = np.zeros((cores, ne), dtype=np.int64)
    dstloc = np.full((cores, ne), -1.0, dtype=np.float64)
    for c in range(cores):
        pos = 0
        for t in range(ndt):
            s_t, dl_t = per[c][t]
            k = len(s_t)
            srcw[c, pos:pos + k] = remap_tab[s_t]
            dstw[c, pos:pos + k] = remap_tab[c * npc + t * P + dl_t]
            dstloc[c, pos:pos + k] = dl_t
            pos += slot_tiles[t] * P
    return dict(npc=npc, ndt=ndt, scs=scs, nt=nt, ne=ne,
                tile2dst=tile2dst, first=first, last=last,
                srcw=srcw, dstw=dstw, dstloc=dstloc)


def _wrap_idx(idx):
    """[NE] -> wrapped int16 [128, NE/16] (16-partition wrap, replicated x8)."""
    ne = idx.shape[0]
    assert ne % 16 == 0
    a = idx.reshape(ne // 16, 16).T.astype(np.int16)
    return np.ascontiguousarray(np.tile(a, (8, 1)))


def _aug_weights(g, heads, din, dout):
    W = np.asarray(g["W"], np.float32)
    a_src = np.asarray(g["a_src"], np.float32)
    a_dst = np.asarray(g["a_dst"], np.float32)
    Wr = W.reshape(din, heads, dout)
    vsrc = np.einsum("fhd,hd->fh", Wr, a_src).astype(np.float32)
    vdst = np.einsum("fhd,hd->fh", Wr, a_dst).astype(np.float32)
    Waug = np.concatenate([W, vsrc, vdst], axis=1)
    kt = math.ceil(din / P)
    pad = np.zeros((kt * P - din, Waug.shape[1]), np.float32)
    Waug = np.concatenate([Waug, pad], 0)
    return np.ascontiguousarray(Waug.reshape(kt, P, Waug.shape[1]))


def _mlp_weights(heads_params):
    """Three combined matrices for the 8 slot-head MLPs.

    m1 = relu(g @ W1) [64, s1tot]; m2 = g1 @ W2 (relu on ternary slice);
    out = m2' @ W3.  W2/W3 are block-diagonal (identity passthrough for
    2-layer heads in W3).
    """
    sizes1 = [np.asarray(l[0][0]).shape[1] for l in heads_params]  # first-layer widths
    s1tot = sum(sizes1)
    W1 = np.zeros((P, s1tot), np.float32)
    col = 0
    for l, s in zip(heads_params, sizes1):
        W1[:np.asarray(l[0][0]).shape[0], col:col + s] = np.asarray(l[0][0], np.float32)
        col += s

    # second stage: per head either final (2-layer head) or middle (3-layer).
    # 3-layer heads are placed FIRST along the m2 feature axis so the relu
    # slice starts at partition 0 (partition offsets must be multiples of 32).
    sizes2 = [np.asarray(l[1][0]).shape[1] for l in heads_params]
    s2tot = sum(sizes2)
    order = [i for i, l in enumerate(heads_params) if len(l) == 3] + \
            [i for i, l in enumerate(heads_params) if len(l) != 3]
    row_off = np.cumsum([0] + sizes1)  # into m1 (head order)
    c2_off = {}
    c = 0
    for i in order:
        c2_off[i] = c
        c += sizes2[i]
    W2 = np.zeros((s1tot, s2tot), np.float32)
    relu_cols = []
    for i, l in enumerate(heads_params):
        W2[row_off[i]:row_off[i] + sizes1[i],
           c2_off[i]:c2_off[i] + sizes2[i]] = np.asarray(l[1][0], np.float32)
        if len(l) == 3:
            relu_cols.append((c2_off[i], c2_off[i] + sizes2[i]))

    sizes3 = [(np.asarray(l[2][0]).shape[1] if len(l) == 3 else s2)
              for l, s2 in zip(heads_params, sizes2)]
    s3tot = sum(sizes3)
    c3_off = np.cumsum([0] + sizes3)  # output stays in head order
    W3 = np.zeros((s2tot, s3tot), np.float32)
    for i, l in enumerate(heads_params):
        r, c = c2_off[i], c3_off[i]
        if len(l) == 3:
            W3[r:r + sizes2[i], c:c + sizes3[i]] = np.asarray(l[2][0], np.float32)
        else:
            W3[r:r + sizes2[i], c:c + sizes3[i]] = np.eye(sizes2[i], dtype=np.float32)

    kt2 = math.ceil(s1tot / P)
    W2p = np.zeros((kt2, P, s2tot), np.float32)
    for k in range(kt2):
        W2p[k, :min(P, s1tot - k * P), :] = W2[k * P:(k + 1) * P, :]
    W3p = np.zeros((P, s3tot), np.float32)
    W3p[:s2tot] = W3
    # relu rows of the m2 intermediate (feature-major partitions)
    assert relu_cols, "expected at least one 3-layer head"
    rlo = min(a for a, _ in relu_cols)
    rhi = max(b for _, b in relu_cols)
    for l in heads_params:
        for _, b, _ in l:
            assert not np.any(np.asarray(b)), "nonzero MLP bias unsupported"
    return W1, W2p, W3p, s1tot, s2tot, s3tot, (rlo, rhi)


# ------------------------------------------------------------- program side --

def _build_program(meta, cores, cfgs, n_graphs, mlp_shapes):
    """Emit the full Tile program; returns (nc, input tensor names)."""
    from concourse import bass, mybir, tile
    from concourse import bacc
    from concourse.masks import make_identity
    from contextlib import ExitStack

    dt = mybir.dt
    f32, bf16, i16 = dt.float32, dt.bfloat16, dt.int16
    OP = mybir.AluOpType
    ACT = mybir.ActivationFunctionType

    npc, ndt, nt, ne = meta["npc"], meta["ndt"], meta["nt"], meta["ne"]
    tile2dst, tfirst, tlast = meta["tile2dst"], meta["first"], meta["last"]
    s1tot, s2tot, s3tot, (rlo, rhi) = mlp_shapes
    rows_l = [1152, 640, 256]  # bf16 table row: fout | f32-bitcast al | pad

    nc = bacc.Bacc("TRN2", target_bir_lowering=False, debug=False,
                   num_devices=cores)
    groups = [list(range(cores))]

    # ---- external inputs
    kt1 = cfgs[0]["kt"]
    n_total = npc * cores
    nft = math.ceil(n_total / P)  # layer-0 stage is replicated over all nodes
    xT_d = nc.dram_tensor("xT", [kt1, nft, P, P], f32, kind="ExternalInput")
    w_d = [nc.dram_tensor(f"w{i+1}", [c["kt"], P, c["fout"] + 2 * c["h"]], f32,
                          kind="ExternalInput") for i, c in enumerate(cfgs)]
    srcw_d = nc.dram_tensor("srcw", [P, ne // 16], i16, kind="ExternalInput")
    dstw_d = nc.dram_tensor("dstw", [P, ne // 16], i16, kind="ExternalInput")
    dstloc_d = nc.dram_tensor("dstloc", [P, nt], f32, kind="ExternalInput")
    iota_d = nc.dram_tensor("iota", [P, P], bf16, kind="ExternalInput")
    spool_d = nc.dram_tensor("spool", [ndt, P, n_graphs], f32, kind="ExternalInput")
    wm1_d = nc.dram_tensor("wm1", [P, s1tot], f32, kind="ExternalInput")
    kt2 = math.ceil(s1tot / P)
    wm2_d = nc.dram_tensor("wm2", [kt2, P, s2tot], f32, kind="ExternalInput")
    wm3_d = nc.dram_tensor("wm3", [P, s3tot], f32, kind="ExternalInput")
    out_d = nc.dram_tensor("out_T", [s3tot, n_graphs], f32, kind="ExternalOutput")

    in_names = ["xT", "w1", "w2", "w3", "srcw", "dstw", "dstloc", "iota",
                "spool", "wm1", "wm2", "wm3"]

    with tile.TileContext(nc) as tc, ExitStack() as ctx:
        dram = ctx.enter_context(tc.tile_pool(name="dram", bufs=1, space="DRAM"))
        tables = [dram.tile([n_total, rows_l[i]], bf16, tag=f"tab{i}",
                            name=f"tab{i}",
                            addr_space="Shared" if (cores > 4 and i > 0)
                            else "Local")
                  for i, c in enumerate(cfgs)]
        shards = [None] + [dram.tile([npc, rows_l[i]], bf16, tag=f"shard{i}",
                                     name=f"shard{i}")
                           for i, c in list(enumerate(cfgs))[1:]]
        g_in = dram.tile([n_graphs, P], f32, tag="g_in")
        g_out = dram.tile([n_graphs, P], f32, tag="g_out")

        cpool = ctx.enter_context(tc.tile_pool(name="const", bufs=1))
        ident = cpool.tile([P, P], f32, tag="ident")
        make_identity(nc, ident[:])
        iota_sb = cpool.tile([P, P], bf16, tag="iota")
        nc.sync.dma_start(iota_sb[:], iota_d.ap())
        srcw = cpool.tile([P, ne // 16], i16, tag="srcw")
        nc.sync.dma_start(srcw[:], srcw_d.ap())
        dstw = cpool.tile([P, ne // 16], i16, tag="dstw")
        nc.sync.dma_start(dstw[:], dstw_d.ap())
        dstloc = cpool.tile([P, nt], f32, tag="dstloc")
        nc.sync.dma_start(dstloc[:], dstloc_d.ap())
        spool_sb = cpool.tile([P, ndt, n_graphs], f32, tag="spool")
        nc.sync.dma_start(spool_sb[:], spool_d.ap().rearrange("d p g -> p d g"))
        wm1_sb = cpool.tile([P, s1tot], f32, tag="wm1")
        nc.sync.dma_start(wm1_sb[:], wm1_d.ap())
        wm2_sb = cpool.tile([P, kt2, s2tot], f32, tag="wm2")
        nc.sync.dma_start(wm2_sb[:], wm2_d.ap().rearrange("k p n -> p k n"))
        wm3_sb = cpool.tile([P, s3tot], f32, tag="wm3")
        nc.sync.dma_start(wm3_sb[:], wm3_d.ap())

        hpool = ctx.enter_context(tc.tile_pool(name="hsb", bufs=1))
        hsb = [hpool.tile([P, ndt, c["fout"]], f32, tag=f"hsb{i}", name=f"hsb{i}")
               for i, c in enumerate(cfgs)]

        sctx = ExitStack()
        sp = sctx.enter_context(tc.tile_pool(name="stg", bufs=2))
        swp = sctx.enter_context(tc.tile_pool(name="stgw", bufs=1))
        spp = sctx.enter_context(tc.tile_pool(name="stgp", bufs=2, space="PSUM"))
        for L, c in enumerate(cfgs):
            h, fout, ktl = c["h"], c["fout"], c["kt"]
            naug = fout + 2 * h
            if True:
                # ---------- stage: h_aug matmuls + shard writes + allgather
                wsb_l = swp.tile([P, ktl, naug], f32, tag="wsb_l",
                                 name=f"wsb_l{L}")
                nc.sync.dma_start(wsb_l[:], w_d[L].ap().rearrange("k p n -> p k n"))
                # walrus requires fp32r matmul operands to be explicitly
                # rounded by their producing instruction
                wsb_r = swp.tile([P, ktl, naug], dt.float32r, tag="wsb_r",
                                 name=f"wsb_r{L}")
                nc.vector.tensor_copy(wsb_r[:], wsb_l[:])
                # L0 is replicated over all node tiles (no collective); L>0
                # computes its own shard then one AllGather.
                row = rows_l[L]
                dest = tables[0] if L == 0 else shards[L]
                n_rows = n_total if L == 0 else npc
                n_dt = nft if L == 0 else ndt
                for d in range(n_dt):
                    dn = min(n_rows - d * P, P)
                    lt = sp.tile([P, ktl, P], dt.float32r, tag="lhs")
                    if L == 0:
                        lt0 = sp.tile([P, ktl, P], f32, tag="lhs0")
                        nc.sync.dma_start(
                            lt0[:], xT_d.ap()[:, d, :, :].rearrange("k p q -> p k q"))
                        nc.vector.tensor_copy(lt[:], lt0[:])
                    else:
                        for k in range(ktl):
                            pt = spp.tile([P, P], f32, tag="ptr")
                            nc.tensor.transpose(
                                out=pt[:], identity=ident[:],
                                in_=hsb[L - 1][:, d, k * P:(k + 1) * P])
                            nc.vector.tensor_copy(lt[:, k, :], pt[:])
                    lhs = [lt[:, k, :] for k in range(ktl)]
                    nsplits = []
                    c0 = 0
                    while c0 < naug:
                        c1 = min(c0 + 512, naug)
                        if c0 < fout < c1:
                            c1 = fout
                        nsplits.append((c0, c1))
                        c0 = c1
                    stf = sp.tile([P, row], bf16, tag="stb")
                    for (c0, c1) in nsplits:
                        ps = spp.tile([P, 512], f32, tag="pstage")
                        for k in range(ktl):
                            nc.tensor.matmul(
                                ps[:, :c1 - c0], lhsT=lhs[k],
                                rhs=wsb_r[:, k, c0:c1],
                                start=(k == 0), stop=(k == ktl - 1))
                        if c1 <= fout:
                            nc.scalar.activation(stf[:, c0:c1], ps[:, :c1 - c0],
                                                 ACT.Copy)
                        else:
                            # al cols: f32 bitcast into the bf16 row + zero pad
                            nc.vector.memset(stf[:, fout + 4 * h:row], 0)
                            nc.vector.tensor_copy(
                                stf[:, fout:fout + 4 * h].bitcast(f32),
                                ps[:, :2 * h])
                    nc.sync.dma_start(dest[d * P:d * P + dn, :], stf[:dn, :])
                if L > 0:
                    nc.gpsimd.collective_compute(
                        "AllGather", mybir.AluOpType.bypass, groups,
                        ins=[shards[L][:, :].opt()],
                        outs=[tables[L][:, :].opt()])

            # ---------- attention
            with tc.tile_pool(name=f"att{L}", bufs=3) as ap, \
                 tc.tile_pool(name=f"attw{L}", bufs=3) as wp, \
                 tc.tile_pool(name=f"aw{L}", bufs=3) as awp, \
                 tc.tile_pool(name=f"attp{L}", bufs=1, space="PSUM") as pp, \
                 tc.tile_pool(name=f"attps{L}", bufs=1, space="PSUM") as pps, \
                 tc.tile_pool(name=f"attps2{L}", bufs=1, space="PSUM") as pps1:
                npacks = math.ceil(h * P / 512)
                packs = None
                den = None
                row = rows_l[L]
                chl = CH  # 1024 descs/call, proven safe on HW
                nchunks = nt // chl
                for ch in range(nchunks):
                    i0 = ch * chl * 8  # wrapped idx col offset
                    g = ap.tile([P, chl, row], bf16, tag="g")
                    nc.gpsimd.dma_gather(
                        g[:], tables[L][:, :], srcw[:, i0:i0 + chl * 8],
                        num_idxs=chl * P, num_idxs_reg=chl * P, elem_size=row)
                    adg = ap.tile([P, chl, P], bf16, tag="adg")
                    nc.gpsimd.dma_gather(
                        adg[:], tables[L][:, fout:fout + P],
                        dstw[:, i0:i0 + chl * 8],
                        num_idxs=chl * P, num_idxs_reg=chl * P, elem_size=P,
                        elem_step=row)
                    # al sections are f32 bitcast inside the bf16 rows
                    wf = wp.tile([P, chl, h], f32, tag="wf")
                    nc.vector.tensor_tensor(
                        out=wf[:],
                        in0=g[:, :, fout:fout + 4 * h].bitcast(f32)[:, :, 0:h],
                        in1=adg[:, :, 0:4 * h].bitcast(f32)[:, :, h:2 * h],
                        op=OP.add)
                    lrt = wp.tile([P, chl, h], f32, tag="lrt")
                    nc.vector.tensor_scalar(
                        out=lrt[:], in0=wf[:], scalar1=0.2, scalar2=None,
                        op0=OP.mult)
                    nc.vector.tensor_tensor(
                        out=wf[:], in0=wf[:], in1=lrt[:], op=OP.max)
                    nc.scalar.activation(wf[:], wf[:], ACT.Exp)
                    wb = wp.tile([P, chl, h], bf16, tag="wb")
                    nc.vector.tensor_copy(wb[:], wf[:])
                    for t in range(chl):
                        gt = ch * chl + t
                        Ab = awp.tile([P, P], bf16, tag="Ab", name="Ab")
                        nc.vector.tensor_scalar(
                            out=Ab[:], in0=iota_sb[:],
                            scalar1=dstloc[:, gt:gt + 1], scalar2=None,
                            op0=OP.is_equal)
                        Aw = awp.tile([P, h, P], bf16, tag="Aw")
                        for hh in range(h):
                            nc.vector.tensor_scalar(
                                out=Aw[:, hh, :], in0=iota_sb[:],
                                scalar1=dstloc[:, gt:gt + 1],
                                scalar2=wf[:, t, hh:hh + 1],
                                op0=OP.is_equal, op1=OP.mult)
                        dtile = tile2dst[gt]
                        if tfirst[gt]:
                            packs = [pp.tile([P, 512], f32, tag=f"pk{i}", name=f"pk{i}")
                                     for i in range(npacks)]
                            den = pps.tile([h, P], f32, tag="den")
                        for hh in range(h):
                            pk = packs[(hh * P) // 512]
                            off = (hh * P) % 512
                            # start=True resets has_written for the whole
                            # PSUM tile -- only the first write to each pack
                            # may set it.
                            nc.tensor.matmul(
                                pk[:, off:off + P], lhsT=Aw[:, hh, :],
                                rhs=g[:, t, hh * P:(hh + 1) * P],
                                start=(tfirst[gt] and off == 0),
                                stop=tlast[gt],
                                skip_group_check=True)
                        nc.tensor.matmul(
                            den[:, :], lhsT=wb[:, t, :], rhs=Ab[:],
                            start=tfirst[gt], stop=tlast[gt],
                            skip_group_check=True)
                        if tlast[gt]:
                            # epilogue for dst tile `dtile`
                            dsb = wp.tile([h, P], f32, tag="dsb")
                            nc.vector.tensor_copy(dsb[:], den[:, :])
                            dtp = pps1.tile([P, h], f32, tag="dtp")
                            nc.tensor.transpose(out=dtp[:], in_=dsb[:],
                                                identity=ident[:h, :h])
                            r = wp.tile([P, h], f32, tag="rcp")
                            nc.vector.tensor_scalar(
                                out=r[:], in0=dtp[:], scalar1=1e-16,
                                scalar2=None, op0=OP.add)
                            nc.vector.reciprocal(r[:], r[:])
                            hv = hsb[L][:, dtile, :]
                            for hh in range(h):
                                pk = packs[(hh * P) // 512]
                                off = (hh * P) % 512
                                nc.scalar.activation(
                                    hv[:, hh * P:(hh + 1) * P],
                                    pk[:, off:off + P], ACT.Copy,
                                    scale=r[:, hh:hh + 1])
                            # elu(v) = max(v, min(exp(v), 1) - 1)
                            tmp = wp.tile([P, fout], f32, tag="elu")
                            nc.scalar.activation(tmp[:], hv, ACT.Exp)
                            nc.vector.tensor_scalar(
                                out=tmp[:], in0=tmp[:], scalar1=1.0,
                                scalar2=-1.0, op0=OP.min, op1=OP.add)
                            nc.vector.tensor_tensor(
                                out=hv, in0=hv, in1=tmp[:], op=OP.max)

        # ---------- mean pool + heads
        sctx.close()  # release stage pools; head phase needs the PSUM banks
        with tc.tile_pool(name="head", bufs=1) as hp, \
             tc.tile_pool(name="headp", bufs=1, space="PSUM") as hpp:
            psg = hpp.tile([n_graphs, P], f32, tag="psg")
            for d in range(ndt):
                nc.tensor.matmul(psg[:], lhsT=spool_sb[:, d, :],
                                 rhs=hsb[2][:, d, :],
                                 start=(d == 0), stop=(d == ndt - 1))
            gsb = hp.tile([n_graphs, P], f32, tag="gsb")
            nc.vector.tensor_copy(gsb[:], psg[:])
            nc.sync.dma_start(g_in[:, :], gsb[:])
            nc.gpsimd.collective_compute(
                "AllReduce", mybir.AluOpType.add, groups,
                ins=[g_in[:, :].opt()], outs=[g_out[:, :].opt()])
            gfull = hp.tile([n_graphs, P], f32, tag="gfull")
            nc.sync.dma_start(gfull[:], g_out[:, :])
            pgt = hpp.tile([P, n_graphs], f32, tag="pgt")
            nc.tensor.transpose(out=pgt[:], in_=gfull[:],
                                identity=ident[:n_graphs, :n_graphs])
            gT = hp.tile([P, n_graphs], f32, tag="gT")
            nc.vector.tensor_copy(gT[:], pgt[:])

            m1 = hp.tile([P, kt2, n_graphs], f32, tag="m1")
            nc.vector.memset(m1[:], 0.0)
            for k in range(kt2):
                mwid = min(P, s1tot - k * P)
                pm = hpp.tile([P, n_graphs], f32, tag="pm1")
                nc.tensor.matmul(pm[:mwid, :], lhsT=wm1_sb[:, k * P:k * P + mwid],
                                 rhs=gT[:], start=True, stop=True)
                nc.scalar.activation(m1[:mwid, k, :], pm[:mwid, :], ACT.Relu)
            pm2 = hpp.tile([s2tot, n_graphs], f32, tag="pm2")
            for k in range(kt2):
                nc.tensor.matmul(pm2[:], lhsT=wm2_sb[:, k, :], rhs=m1[:, k, :],
                                 start=(k == 0), stop=(k == kt2 - 1))
            m2 = hp.tile([P, n_graphs], f32, tag="m2")
            nc.vector.memset(m2[:], 0.0)
            if rlo > 0:
                nc.vector.tensor_copy(m2[0:rlo, :], pm2[0:rlo, :])
            nc.scalar.activation(m2[rlo:rhi, :], pm2[rlo:rhi, :], ACT.Relu)
            if rhi < s2tot:
                nc.vector.tensor_copy(m2[rhi:s2tot, :], pm2[rhi:s2tot, :])
            pm3 = hpp.tile([s3tot, n_graphs], f32, tag="pm3")
            nc.tensor.matmul(pm3[:], lhsT=wm3_sb[:], rhs=m2[:], start=True,
                             stop=True)
            osb = hp.tile([s3tot, n_graphs], f32, tag="osb")
            nc.vector.tensor_copy(osb[:], pm3[:])
            nc.sync.dma_start(out_d.ap(), osb[:])

    nc.compile()
    return nc, in_names


# ------------------------------------------------------------------ driver --

def _host_inputs(x, edge_index, batch, params, meta, cfgs, cores):
    """Per-core input tensors for the program."""
    import ml_dtypes

    x = np.asarray(x, np.float32)
    batch = np.asarray(batch)
    n_nodes = x.shape[0]
    n_graphs = 64
    npc = n_nodes // cores
    ndt = meta["ndt"]

    g1, g2, g3 = params["gat1"], params["gat2"], params["gat3"]
    fin1 = x.shape[1]
    w1 = _aug_weights(g1, 8, fin1, 128)
    w2 = _aug_weights(g2, 4, 1024, 128)
    w3 = _aug_weights(g3, 1, 512, 128)
    for g in (g1, g2, g3):
        assert not np.any(np.asarray(g["b"])), "nonzero GAT bias unsupported"
    W1m, W2m, W3m, *_ = _mlp_weights(params["heads"])

    cnts = np.bincount(batch, minlength=n_graphs).astype(np.float64)
    cnts = np.maximum(cnts, 1.0)
    spool = np.zeros((cores, ndt, P, n_graphs), np.float32)
    inv = 1.0 / cnts[batch]
    for c in range(cores):
        for d in range(ndt):
            pn = min(P, npc - d * P)
            n0 = c * npc + d * P
            spool[c, d, np.arange(pn), batch[n0:n0 + pn]] = inv[n0:n0 + pn]

    kt1 = cfgs[0]["kt"]
    nft = math.ceil(n_nodes / P)
    xT = np.zeros((kt1, nft, P, P), np.float32)
    for k in range(kt1):
        for d in range(nft):
            blk = x[d * P:(d + 1) * P, k * P:(k + 1) * P]
            xT[k, d, :blk.shape[1], :blk.shape[0]] = blk.T

    iota = np.tile(np.arange(P, dtype=np.float32), (P, 1))

    in_maps = []
    for c in range(cores):
        in_maps.append({
            "xT": xT,
            "w1": w1, "w2": w2, "w3": w3,
            "srcw": _wrap_idx(meta["srcw"][c]),
            "dstw": _wrap_idx(meta["dstw"][c]),
            "dstloc": np.ascontiguousarray(
                meta["dstloc"][c].reshape(meta["nt"], P).T.astype(np.float32)),
            "iota": iota.astype(ml_dtypes.bfloat16),
            "spool": np.ascontiguousarray(spool[c]),
            "wm1": W1m, "wm2": W2m, "wm3": W3m,
        })
    return in_maps


def _run(x, edge_index, batch, params, cores=8, trace=False):
    from concourse.bass_utils import run_bass_kernel_spmd

    x = np.asarray(x, np.float32)
    edge_index = np.asarray(edge_index)
    batch = np.asarray(batch)
    n_nodes = x.shape[0]
    n_graphs = 64

    meta = _preprocess(edge_index, n_nodes, cores)
    fin1 = x.shape[1]
    cfgs = [
        dict(h=8, fout=1024, kt=math.ceil(fin1 / P)),
        dict(h=4, fout=512, kt=8),
        dict(h=1, fout=128, kt=4),
    ]
    _, _, _, s1tot, s2tot, s3tot, relu_rows = _mlp_weights(params["heads"])
    nc, _ = _build_program(meta, cores, cfgs, n_graphs,
                           (s1tot, s2tot, s3tot, relu_rows))
    in_maps = _host_inputs(x, edge_index, batch, params, meta, cfgs, cores)
    res = run_bass_kernel_spmd(nc, in_maps, core_ids=list(range(cores)),
                               trace=trace)
    out = res.results[0]["out_T"]  # [18, 64]
    return np.ascontiguousarray(out.T.astype(np.float32)), res


def kernel(x, edge_index, batch, params):
    out, _ = _run(x, edge_index, batch, params)
    return out
